# revision 1
# baseline (speedup 1.0000x reference)
"""GAT (2-layer, DGL-style) Bass kernel for Trainium2, 8-core SPMD. v2.

Strategy (dst-sharded, edge-parallel within core):
- Node rows sharded across 8 cores by dst range (6250/core).
- GEMM1 folds el/er attention dots into extended W columns; row = 544
  [msg 512 | el 8 | er 8 | pad 16]; AllGather -> replicated hfull.
- Edge phase per 128-dst-block group: src rows fetched with dma_gather
  (the only descriptor stream); er per edge computed ON-CHIP via a
  transposed one-hot (ohT[d,e]) matmul against the block's er rows --
  no er gather stream (halves DMA descriptors vs v1).
- Segment softmax numerator/denominator accumulate per block with
  one-hot matmuls into PSUM (exp(e) written into the slab el slot so
  one rhs covers msg[256:512]+a).
- GEMM2 fused per block (TensorE transpose + matmul) -> bounce_h2.
- No segment-max subtraction (reference's emax cancels exactly).
"""

import sys
import time
import hashlib

for _p in ("/opt/trn_rl_repo", "/root/.axon_site", "/root/.axon_site/_ro/trn_rl_repo"):
    if _p not in sys.path:
        sys.path.insert(0, _p)

import numpy as np
import ml_dtypes

import concourse.bass as bass
import concourse.mybir as mybir
import concourse.tile as tile
from concourse import bacc
from concourse import bass_utils

F32 = mybir.dt.float32
BF16 = mybir.dt.bfloat16
I16 = mybir.dt.int16
BF = ml_dtypes.bfloat16

NEG_SLOPE = 0.2
DENOM_EPS = 1e-30


# ---------------------------------------------------------------------------
# configuration


class Cfg:
    def __init__(self, N, E, F, HID, HEADS, NC=8, GBLK=2):
        self.N, self.E, self.F, self.HID, self.HEADS, self.NC = N, E, F, HID, HEADS, NC
        self.HALF = N // 2
        self.NB = N // NC                      # nodes per core
        self.NBLK = (self.NB + 127) // 128     # dst blocks per core
        self.NBpad = self.NBLK * 128
        self.GBLK = GBLK
        self.KT1 = (F + 127) // 128
        self.Fpad = self.KT1 * 128
        self.C1 = HEADS * HID                  # 512
        self.KT2 = (self.C1 + 127) // 128      # 4
        self.C1pad = self.KT2 * 128
        self.C2 = HID                          # 64
        # layer-1 row: [msg 512 | el 8 | er 8 | pad] -> 544 (1088 B, %256==64?
        # no: dma_gather needs elem bytes %256==0 -> 1088 % 256 = 64. BAD.
        # Use 640 elems (1280 B) like v1: [msg 512|el 8|er 8|pad 112].
        self.RW1 = 640
        self.EL1 = self.C1                     # el offset
        self.ER1 = self.C1 + HEADS             # er offset
        # layer-2 row: [msg 64 | el 1 | er 1 | pad] -> 128 (256 B)
        self.RW2 = 128
        self.EL2 = self.C2
        self.ER2 = self.C2 + 1
        assert self.HALF < 32768 and self.NB < 32768


# ---------------------------------------------------------------------------
# host-side edge preprocessing


def wrap_idxs(idx: np.ndarray) -> np.ndarray:
    """int16 idx list (len = nt*128) -> [128, nt*8] wrapped-16, replicated."""
    n = len(idx)
    assert n % 128 == 0
    m = n // 16
    out = np.zeros((16, m), dtype=np.int16)
    out[np.arange(n) % 16, np.arange(n) // 16] = idx
    return np.tile(out, (8, 1))


def make_plan(src, dst, cfg: Cfg):
    """Static tile plan (shared across cores) + per-core edge tensors."""
    NC, NB, NBLK, HALF, GBLK = cfg.NC, cfg.NB, cfg.NBLK, cfg.HALF, cfg.GBLK

    per_core = []
    counts = np.zeros((NC, NBLK, 2), dtype=np.int64)
    for r in range(NC):
        sel = np.nonzero((dst >= r * NB) & (dst < (r + 1) * NB))[0]
        es = src[sel].astype(np.int64)
        ed = (dst[sel] - r * NB).astype(np.int64)
        blk = ed >> 7
        half = (es >= HALF).astype(np.int64)
        key = blk * 2 + half
        o = np.argsort(key, kind="stable")
        es, ed, key = es[o], ed[o], key[o]
        cnt = np.bincount(key, minlength=NBLK * 2)
        counts[r] = cnt.reshape(NBLK, 2)
        per_core.append((es, ed, np.concatenate([[0], np.cumsum(cnt)])))

    cmax = counts.max(axis=0)                  # [NBLK, 2]
    ntile = (cmax + 127) // 128

    groups = []
    for g0 in range(0, NBLK, GBLK):
        blocks = list(range(g0, min(g0 + GBLK, NBLK)))
        ntlo = int(sum(ntile[b, 0] for b in blocks))
        nthi = int(sum(ntile[b, 1] for b in blocks))
        nt = ntlo + nthi
        tile_blk = []
        for b in blocks:
            tile_blk += [b] * int(ntile[b, 0])
        for b in blocks:
            tile_blk += [b] * int(ntile[b, 1])
        groups.append(dict(blocks=blocks, ntlo=ntlo, nthi=nthi, nt=nt,
                           tile_blk=tile_blk))

    # per-core flat arrays in group/tile order
    core_data = []
    for r in range(NC):
        es, ed, cum = per_core[r]
        lo_idx, hi_idx, dvals = [], [], []
        for g in groups:
            for h, acc in ((0, lo_idx), (1, hi_idx)):
                for b in g["blocks"]:
                    k = b * 2 + h
                    s, e = cum[k], cum[k + 1]
                    n_pad = int(ntile[b, h]) * 128
                    v = es[s:e] - (HALF if h else 0)
                    v = np.concatenate([v, np.zeros(n_pad - len(v), np.int64)])
                    acc.append(v)
                    d = np.concatenate([ed[s:e],
                                        np.full(n_pad - (e - s), -1, np.int64)])
                    dv = np.where(d >= 0, d & 127, 999).astype(np.float32)
                    dvals.append(dv)
        lo = np.concatenate(lo_idx) if lo_idx else np.zeros(0, np.int64)
        hi = np.concatenate(hi_idx) if hi_idx else np.zeros(0, np.int64)
        dv = np.concatenate(dvals)
        # wrap per group; dl in [128, nt] layout; dlT replicated [128, nt*128]
        glo, ghi, gdl, gdt = [], [], [], []
        plo = phi = pall = 0
        for g in groups:
            nlo, nhi, nt = g["ntlo"] * 128, g["nthi"] * 128, g["nt"] * 128
            glo.append(wrap_idxs(lo[plo:plo + nlo].astype(np.int16)))
            ghi.append(wrap_idxs(hi[phi:phi + nhi].astype(np.int16)))
            dvg = dv[pall:pall + nt]
            gdl.append(dvg.reshape(g["nt"], 128).T)
            gdt.append(np.tile(dvg.reshape(1, nt), (128, 1)))
            plo, phi, pall = plo + nlo, phi + nhi, pall + nt
        core_data.append(dict(
            g1lo=np.concatenate(glo, axis=1) if glo else np.zeros((128, 0), np.int16),
            g1hi=np.concatenate(ghi, axis=1) if ghi else np.zeros((128, 0), np.int16),
            dstloc=np.concatenate(gdl, axis=1).astype(BF),
            dstlocT=np.concatenate(gdt, axis=1).astype(BF),
        ))

    plan = dict(groups=groups, ntile=ntile,
                TLO=int(sum(g["ntlo"] for g in groups)),
                THI=int(sum(g["nthi"] for g in groups)),
                TT=int(sum(g["nt"] for g in groups)))
    return plan, core_data


MAX_GIDX = 1024  # HW limit: dma_gather crashes above 1024 indices


def chunked_gather(nc, out_slab, in_ap, idx_tile, t0, ntc, elem, step=None):
    CT = MAX_GIDX // 128
    for q0 in range(0, ntc, CT):
        qn = min(CT, ntc - q0)
        kw = dict(elem_step=step) if step else {}
        nc.gpsimd.dma_gather(
            out_ap=out_slab[:, t0 + q0:t0 + q0 + qn, :], in_ap=in_ap,
            idxs_ap=idx_tile[:, q0 * 8:(q0 + qn) * 8],
            num_idxs=qn * 128, num_idxs_reg=qn * 128, elem_size=elem, **kw)


# ---------------------------------------------------------------------------
# program builder


def build_program(cfg: Cfg, plan, reps=1, phases="ABCEF"):
    ag_space = "Shared" if reps == 1 else "Local"
    c = cfg
    H = c.HEADS
    nc = bacc.Bacc("TRN2", target_bir_lowering=False, debug=False,
                   num_devices=c.NC)

    dt = nc.dram_tensor
    featsT = dt("featsT", [c.KT1, 128, c.NBpad], BF16, kind="ExternalInput")
    w1ext = dt("w1ext", [c.KT1, 128, c.RW1], BF16, kind="ExternalInput")
    w2ext = dt("w2ext", [c.KT2, 128, c.RW2], BF16, kind="ExternalInput")
    b1rep = dt("b1rep", [128, c.C1], F32, kind="ExternalInput")
    b2rep = dt("b2rep", [128, c.C2], F32, kind="ExternalInput")
    iota = dt("iota", [128, 128], BF16, kind="ExternalInput")
    iotap = dt("iotap", [128, 1], BF16, kind="ExternalInput")
    g1lo = dt("g1lo", [128, max(1, plan["TLO"] * 8)], I16, kind="ExternalInput")
    g1hi = dt("g1hi", [128, max(1, plan["THI"] * 8)], I16, kind="ExternalInput")
    dstloc = dt("dstloc", [128, plan["TT"]], BF16, kind="ExternalInput")
    dstlocT = dt("dstlocT", [128, plan["TT"] * 128], BF16, kind="ExternalInput")
    out = dt("out", [c.NB, c.C2], F32, kind="ExternalOutput")

    groups = plan["groups"]

    with tile.TileContext(nc) as tc:
        with tc.tile_pool(name="dram", bufs=1, space="DRAM") as dram:
            bounce_h = dram.tile([c.NB, c.RW1], BF16)
            hfull = dram.tile([c.N, c.RW1], BF16, addr_space=ag_space)
            bounce_h2 = dram.tile([c.NB, c.RW2], BF16)
            h2full = dram.tile([c.N, c.RW2], BF16, addr_space=ag_space)

            for _rep in range(reps):
                if "A" in phases:
                    # ------------- phase A: GEMM1 -> bounce_h -------------
                    with (
                        nc.named_scope("phaseA_gemm1"),
                        tc.tile_pool(name="ga", bufs=1) as cpool,
                        tc.tile_pool(name="gaw", bufs=2) as wpool,
                        tc.tile_pool(name="gap", bufs=2, space="PSUM") as pspool,
                    ):
                        w1sb = cpool.tile([128, c.KT1, c.RW1], BF16)
                        ftsb = cpool.tile([128, c.KT1, c.NBpad], BF16)
                        nc.sync.dma_start(w1sb[:], w1ext[:].rearrange("k p w -> p k w"))
                        nc.sync.dma_start(ftsb[:], featsT[:].rearrange("k p w -> p k w"))
                        for ntb in range(c.NBLK):
                            pa = pspool.tile([128, 512], F32, tag="pa", space="PSUM")
                            pb = pspool.tile([128, c.RW1 - 512], F32, tag="pb",
                                             space="PSUM")
                            for k in range(c.KT1):
                                lhsT = ftsb[:, k, ntb * 128:(ntb + 1) * 128]
                                nc.tensor.matmul(pa[:], lhsT, w1sb[:, k, 0:512],
                                                 start=(k == 0), stop=(k == c.KT1 - 1))
                                nc.tensor.matmul(pb[:], lhsT,
                                                 w1sb[:, k, 512:c.RW1],
                                                 start=(k == 0), stop=(k == c.KT1 - 1))
                            ht = wpool.tile([128, c.RW1], BF16, tag="ht")
                            nc.vector.tensor_copy(ht[:, 0:512], pa[:])
                            nc.vector.tensor_copy(ht[:, 512:c.RW1], pb[:])
                            rows = min(128, c.NB - ntb * 128)
                            nc.sync.dma_start(bounce_h[ntb * 128:ntb * 128 + rows, :],
                                              ht[:rows, :])

                if "B" in phases:
                    # ------------- phase B: AllGather h -------------
                    with nc.named_scope("phaseB_ag1"):
                        nc.gpsimd.collective_compute(
                            "AllGather", mybir.AluOpType.bypass,
                            replica_groups=[list(range(c.NC))],
                            ins=[bounce_h.opt()], outs=[hfull.opt()],
                        )

                if "C" in phases:
                    # ------- phase C: layer-1 edge phase + fused GEMM2 -------
                    with (
                        nc.named_scope("phaseC_edge1"),
                        tc.tile_pool(name="ec", bufs=1) as cst,
                        tc.tile_pool(name="e1", bufs=2) as sb,
                        tc.tile_pool(name="e1n", bufs=2) as nsb,
                        tc.tile_pool(name="e1p", bufs=2, space="PSUM") as ps,
                        tc.tile_pool(name="e1e", bufs=1, space="PSUM") as pse,
                        tc.tile_pool(name="e1q", bufs=2, space="PSUM") as psq,
                        tc.tile_pool(name="e1t", bufs=1, space="PSUM") as pst,
                    ):
                        iosb = cst.tile([128, 128], BF16)
                        nc.sync.dma_start(iosb[:], iota[:])
                        iopsb = cst.tile([128, 1], BF16)
                        nc.sync.dma_start(iopsb[:], iotap[:])
                        b1sb = cst.tile([128, c.C1], F32)
                        nc.sync.dma_start(b1sb[:], b1rep[:])
                        b2sb = cst.tile([128, c.C2], F32)
                        nc.sync.dma_start(b2sb[:], b2rep[:])
                        ident = cst.tile([128, 128], BF16)
                        nc.vector.memset(ident[:], 0.0)
                        nc.vector.tensor_tensor(
                            out=ident[:], in0=iosb[:],
                            in1=iopsb[:].broadcast_to([128, 128]),
                            op=mybir.AluOpType.is_equal)
                        w2sb = cst.tile([128, c.KT2, c.RW2], BF16)
                        nc.sync.dma_start(w2sb[:], w2ext[:].rearrange("k p w -> p k w"))
                        # relu(b1) -> xb0; its fused-GEMM2 row (for empty blocks)
                        xb0 = cst.tile([128, c.C1], BF16)
                        nc.scalar.activation(xb0[:], b1sb[:],
                                             mybir.ActivationFunctionType.Relu)
                        xt20 = cst.tile([128, c.KT2, 128], BF16)
                        h2t0 = cst.tile([128, c.RW2], BF16)
                        pt0 = pst.tile([128, 128], BF16, tag="pt", space="PSUM")
                        for k in range(c.KT2):
                            nc.tensor.transpose(
                                pt0[:], xb0[:, k * 128:(k + 1) * 128], ident[:])
                            nc.vector.tensor_copy(xt20[:, k, :], pt0[:])
                        pc20 = psq.tile([128, c.RW2], F32, tag="pc2", space="PSUM")
                        for k in range(c.KT2):
                            nc.tensor.matmul(pc20[:], xt20[:, k, :], w2sb[:, k, :],
                                             start=(k == 0), stop=(k == c.KT2 - 1))
                        nc.vector.tensor_copy(h2t0[:], pc20[:])

                        olo = ohi = oall = 0
                        for g in groups:
                            ntlo, nthi, nt = g["ntlo"], g["nthi"], g["nt"]
                            if nt == 0:
                                continue
                            slab = sb.tile([128, nt, c.RW1], BF16, tag="slab")
                            oh = sb.tile([128, nt, 128], BF16, tag="oh")
                            ohT = sb.tile([128, nt, 128], BF16, tag="ohT")
                            dlt = sb.tile([128, nt * 128], BF16, tag="dlt")
                            dl = sb.tile([128, nt], BF16, tag="dl")
                            nc.sync.dma_start(dl[:], dstloc[:, oall:oall + nt])
                            nc.sync.dma_start(
                                dlt[:], dstlocT[:, oall * 128:(oall + nt) * 128])
                            if ntlo:
                                ilo = sb.tile([128, ntlo * 8], I16, tag="ilo")
                                nc.sync.dma_start(ilo[:], g1lo[:, olo * 8:(olo + ntlo) * 8])
                                chunked_gather(nc, slab, hfull[0:c.HALF, :], ilo,
                                               0, ntlo, c.RW1)
                            if nthi:
                                ihi = sb.tile([128, nthi * 8], I16, tag="ihi")
                                nc.sync.dma_start(ihi[:], g1hi[:, ohi * 8:(ohi + nthi) * 8])
                                chunked_gather(nc, slab, hfull[c.HALF:c.N, :], ihi,
                                               ntlo, nthi, c.RW1)
                            # one-hot [e, d] and transposed one-hot [d, e]
                            nc.vector.tensor_tensor(
                                out=oh[:],
                                in0=iosb[:, None, :].broadcast_to([128, nt, 128]),
                                in1=dl[:, :, None].broadcast_to([128, nt, 128]),
                                op=mybir.AluOpType.is_equal)
                            nc.vector.tensor_tensor(
                                out=ohT[:].rearrange("p t e -> p (t e)"),
                                in0=iopsb[:].broadcast_to([128, nt * 128]),
                                in1=dlt[:],
                                op=mybir.AluOpType.is_equal)
                            # er per edge: erG[:, t, :] = ohT_t^T @ er_block
                            erg = pse.tile([128, nt, H], F32, tag="erg", space="PSUM")
                            erb = sb.tile([128, c.GBLK, H], BF16, tag="erb")
                            nc.vector.memset(erb[:], 0.0)
                            for j, b in enumerate(g["blocks"]):
                                rows = min(128, c.NB - b * 128)
                                nc.sync.dma_start(
                                    erb[:rows, j, :],
                                    bounce_h[b * 128:b * 128 + rows,
                                             c.ER1:c.ER1 + H])
                            for t, tb in enumerate(g["tile_blk"]):
                                j = g["blocks"].index(tb)
                                nc.tensor.matmul(erg[:, t, :], ohT[:, t, :],
                                                 erb[:, j, :],
                                                 start=True, stop=True)
                            # e = el + er ; lrelu ; exp -> a (into el slot)
                            et = sb.tile([128, nt, H], F32, tag="et")
                            e2 = sb.tile([128, nt, H], F32, tag="e2")
                            nc.vector.tensor_tensor(
                                out=et[:], in0=slab[:, :, c.EL1:c.EL1 + H],
                                in1=erg[:], op=mybir.AluOpType.add)
                            nc.vector.tensor_scalar_mul(e2[:], et[:], NEG_SLOPE)
                            nc.vector.tensor_tensor(out=e2[:], in0=e2[:], in1=et[:],
                                                    op=mybir.AluOpType.max)
                            nc.scalar.activation(slab[:, :, c.EL1:c.EL1 + H], e2[:],
                                                 mybir.ActivationFunctionType.Exp)
                            # msg = h * a (in-place, broadcast a over HID)
                            nc.vector.tensor_tensor(
                                out=slab[:, :, 0:c.C1].rearrange(
                                    "p t (h f) -> p t h f", h=H),
                                in0=slab[:, :, 0:c.C1].rearrange(
                                    "p t (h f) -> p t h f", h=H),
                                in1=slab[:, :, c.EL1:c.EL1 + H][:, :, :, None]
                                .broadcast_to([128, nt, H, c.HID]),
                                op=mybir.AluOpType.mult)
                            # per-block accumulate + normalize + fused GEMM2
                            for b in g["blocks"]:
                                tlist = [t for t, tb in enumerate(g["tile_blk"])
                                         if tb == b]
                                rows = min(128, c.NB - b * 128)
                                if not tlist:
                                    nc.sync.dma_start(
                                        bounce_h2[b * 128:b * 128 + rows, :],
                                        h2t0[:rows, :])
                                    continue
                                pa = ps.tile([128, 256], F32, tag="pa", space="PSUM")
                                pb = ps.tile([128, 264], F32, tag="pb", space="PSUM")
                                for j, t in enumerate(tlist):
                                    st, sp = (j == 0), (j == len(tlist) - 1)
                                    nc.tensor.matmul(pa[:], oh[:, t, :],
                                                     slab[:, t, 0:256],
                                                     start=st, stop=sp)
                                    nc.tensor.matmul(pb[:], oh[:, t, :],
                                                     slab[:, t, 256:520],
                                                     start=st, stop=sp)
                                dg = nsb.tile([128, H], F32, tag="dg")
                                rd = nsb.tile([128, H], F32, tag="rd")
                                nc.vector.tensor_scalar_max(dg[:], pb[:, 256:264],
                                                            DENOM_EPS)
                                nc.vector.reciprocal(rd[:], dg[:])
                                xt = nsb.tile([128, c.C1], F32, tag="xt")
                                H2 = H // 2
                                nc.vector.tensor_tensor(
                                    out=xt[:, 0:256].rearrange("p (h f) -> p h f", h=H2),
                                    in0=pa[:].rearrange("p (h f) -> p h f", h=H2),
                                    in1=rd[:, 0:H2][:, :, None]
                                    .broadcast_to([128, H2, c.HID]),
                                    op=mybir.AluOpType.mult)
                                nc.vector.tensor_tensor(
                                    out=xt[:, 256:512].rearrange("p (h f) -> p h f", h=H2),
                                    in0=pb[:, 0:256].rearrange("p (h f) -> p h f", h=H2),
                                    in1=rd[:, H2:H][:, :, None]
                                    .broadcast_to([128, H2, c.HID]),
                                    op=mybir.AluOpType.mult)
                                nc.vector.tensor_tensor(out=xt[:], in0=xt[:],
                                                        in1=b1sb[:],
                                                        op=mybir.AluOpType.add)
                                xb = nsb.tile([128, c.C1], BF16, tag="xb")
                                nc.scalar.activation(xb[:], xt[:],
                                                     mybir.ActivationFunctionType.Relu)
                                # fused GEMM2 for this block
                                xt2 = nsb.tile([128, c.KT2, 128], BF16, tag="xt2")
                                pt = pst.tile([128, 128], BF16, tag="pt", space="PSUM")
                                for k in range(c.KT2):
                                    nc.tensor.transpose(
                                        pt[:], xb[:, k * 128:(k + 1) * 128], ident[:])
                                    nc.vector.tensor_copy(xt2[:, k, :], pt[:])
                                pc2 = psq.tile([128, c.RW2], F32, tag="pc2",
                                               space="PSUM")
                                for k in range(c.KT2):
                                    nc.tensor.matmul(pc2[:], xt2[:, k, :],
                                                     w2sb[:, k, :],
                                                     start=(k == 0),
                                                     stop=(k == c.KT2 - 1))
                                h2t = nsb.tile([128, c.RW2], BF16, tag="h2t")
                                nc.vector.tensor_copy(h2t[:], pc2[:])
                                nc.sync.dma_start(
                                    bounce_h2[b * 128:b * 128 + rows, :],
                                    h2t[:rows, :])
                            olo, ohi, oall = olo + ntlo, ohi + nthi, oall + nt

                if "E" in phases:
                    # ------------- phase E: AllGather h2 -------------
                    with nc.named_scope("phaseE_ag2"):
                        nc.gpsimd.collective_compute(
                            "AllGather", mybir.AluOpType.bypass,
                            replica_groups=[list(range(c.NC))],
                            ins=[bounce_h2.opt()], outs=[h2full.opt()],
                        )

                if "F" in phases:
                    # ------------- phase F: layer-2 edge phase -------------
                    with (
                        nc.named_scope("phaseF_edge2"),
                        tc.tile_pool(name="fc", bufs=1) as cst,
                        tc.tile_pool(name="f1", bufs=2) as sb,
                        tc.tile_pool(name="f1n", bufs=2) as nsb,
                        tc.tile_pool(name="f1p", bufs=2, space="PSUM") as ps,
                    ):
                        iosb = cst.tile([128, 128], BF16)
                        nc.sync.dma_start(iosb[:], iota[:])
                        iopsb = cst.tile([128, 1], BF16)
                        nc.sync.dma_start(iopsb[:], iotap[:])
                        b2sb = cst.tile([128, c.C2], F32)
                        nc.sync.dma_start(b2sb[:], b2rep[:])
                        ot0 = cst.tile([128, c.C2], F32)
                        nc.vector.tensor_copy(ot0[:], b2sb[:])

                        olo = ohi = oall = 0
                        for g in groups:
                            ntlo, nthi, nt = g["ntlo"], g["nthi"], g["nt"]
                            if nt == 0:
                                continue
                            slab = sb.tile([128, nt, c.RW2], BF16, tag="slab2")
                            oh = sb.tile([128, nt, 128], BF16, tag="oh2")
                            ohT = sb.tile([128, nt, 128], BF16, tag="ohT2")
                            dlt = sb.tile([128, nt * 128], BF16, tag="dlt2")
                            dl = sb.tile([128, nt], BF16, tag="dl2")
                            nc.sync.dma_start(dl[:], dstloc[:, oall:oall + nt])
                            nc.sync.dma_start(
                                dlt[:], dstlocT[:, oall * 128:(oall + nt) * 128])
                            if ntlo:
                                ilo = sb.tile([128, ntlo * 8], I16, tag="ilo2")
                                nc.sync.dma_start(ilo[:], g1lo[:, olo * 8:(olo + ntlo) * 8])
                                chunked_gather(nc, slab, h2full[0:c.HALF, :], ilo,
                                               0, ntlo, c.RW2)
                            if nthi:
                                ihi = sb.tile([128, nthi * 8], I16, tag="ihi2")
                                nc.sync.dma_start(ihi[:], g1hi[:, ohi * 8:(ohi + nthi) * 8])
                                chunked_gather(nc, slab, h2full[c.HALF:c.N, :], ihi,
                                               ntlo, nthi, c.RW2)
                            nc.vector.tensor_tensor(
                                out=oh[:],
                                in0=iosb[:, None, :].broadcast_to([128, nt, 128]),
                                in1=dl[:, :, None].broadcast_to([128, nt, 128]),
                                op=mybir.AluOpType.is_equal)
                            nc.vector.tensor_tensor(
                                out=ohT[:].rearrange("p t e -> p (t e)"),
                                in0=iopsb[:].broadcast_to([128, nt * 128]),
                                in1=dlt[:],
                                op=mybir.AluOpType.is_equal)
                            erg = ps.tile([128, nt, 1], F32, tag="erg2", space="PSUM")
                            erb = sb.tile([128, c.GBLK, 1], BF16, tag="erb2")
                            nc.vector.memset(erb[:], 0.0)
                            for j, b in enumerate(g["blocks"]):
                                rows = min(128, c.NB - b * 128)
                                nc.sync.dma_start(
                                    erb[:rows, j, :],
                                    bounce_h2[b * 128:b * 128 + rows,
                                              c.ER2:c.ER2 + 1])
                            for t, tb in enumerate(g["tile_blk"]):
                                j = g["blocks"].index(tb)
                                nc.tensor.matmul(erg[:, t, :], ohT[:, t, :],
                                                 erb[:, j, :],
                                                 start=True, stop=True)
                            et = sb.tile([128, nt, 1], F32, tag="et2")
                            e2 = sb.tile([128, nt, 1], F32, tag="e22")
                            nc.vector.tensor_tensor(
                                out=et[:], in0=slab[:, :, c.EL2:c.EL2 + 1],
                                in1=erg[:], op=mybir.AluOpType.add)
                            nc.vector.tensor_scalar_mul(e2[:], et[:], NEG_SLOPE)
                            nc.vector.tensor_tensor(out=e2[:], in0=e2[:], in1=et[:],
                                                    op=mybir.AluOpType.max)
                            nc.scalar.activation(slab[:, :, c.EL2:c.EL2 + 1], e2[:],
                                                 mybir.ActivationFunctionType.Exp)
                            nc.vector.tensor_tensor(
                                out=slab[:, :, 0:c.C2],
                                in0=slab[:, :, 0:c.C2],
                                in1=slab[:, :, c.EL2:c.EL2 + 1].broadcast_to(
                                    [128, nt, c.C2]),
                                op=mybir.AluOpType.mult)
                            for b in g["blocks"]:
                                tlist = [t for t, tb in enumerate(g["tile_blk"])
                                         if tb == b]
                                rows = min(128, c.NB - b * 128)
                                if not tlist:
                                    nc.sync.dma_start(out[b * 128:b * 128 + rows, :],
                                                      ot0[:rows, :])
                                    continue
                                pc = ps.tile([128, c.C2 + 1], F32, tag="pc",
                                             space="PSUM")
                                for j, t in enumerate(tlist):
                                    nc.tensor.matmul(pc[:], oh[:, t, :],
                                                     slab[:, t, 0:c.C2 + 1],
                                                     start=(j == 0),
                                                     stop=(j == len(tlist) - 1))
                                dg = nsb.tile([128, 1], F32, tag="dg2")
                                rd = nsb.tile([128, 1], F32, tag="rd2")
                                nc.vector.tensor_scalar_max(dg[:], pc[:, c.C2:c.C2 + 1],
                                                            DENOM_EPS)
                                nc.vector.reciprocal(rd[:], dg[:])
                                ot = nsb.tile([128, c.C2], F32, tag="ot")
                                nc.vector.tensor_scalar(
                                    out=ot[:], in0=pc[:, 0:c.C2], scalar1=rd[:, 0:1],
                                    scalar2=None, op0=mybir.AluOpType.mult)
                                nc.vector.tensor_tensor(out=ot[:], in0=ot[:],
                                                        in1=b2sb[:],
                                                        op=mybir.AluOpType.add)
                                nc.sync.dma_start(out[b * 128:b * 128 + rows, :],
                                                  ot[:rows, :])
                            olo, ohi, oall = olo + ntlo, ohi + nthi, oall + nt

    nc.compile()
    return nc


# ---------------------------------------------------------------------------
# host orchestration


def make_inputs(inputs, cfg: Cfg, plan, core_data):
    c = cfg
    feats = np.asarray(inputs["feats"], np.float32)
    W1 = np.asarray(inputs["W1"], np.float32)
    al1 = np.asarray(inputs["attn_l1"], np.float32)
    ar1 = np.asarray(inputs["attn_r1"], np.float32)
    b1 = np.asarray(inputs["b1"], np.float32)
    W2 = np.asarray(inputs["W2"], np.float32)
    al2 = np.asarray(inputs["attn_l2"], np.float32)
    ar2 = np.asarray(inputs["attn_r2"], np.float32)
    b2 = np.asarray(inputs["b2"], np.float32)

    H, HID = c.HEADS, c.HID
    W1r = W1.reshape(c.F, H, HID)
    Wl1 = np.einsum("khd,hd->kh", W1r, al1)
    Wr1 = np.einsum("khd,hd->kh", W1r, ar1)
    w1e = np.zeros((c.Fpad, c.RW1), np.float32)
    w1e[:c.F, 0:c.C1] = W1
    w1e[:c.F, c.EL1:c.EL1 + H] = Wl1
    w1e[:c.F, c.ER1:c.ER1 + H] = Wr1
    w1e = w1e.reshape(c.KT1, 128, c.RW1).astype(BF)

    Wl2 = W2 @ al2[0]
    Wr2 = W2 @ ar2[0]
    w2e = np.zeros((c.C1pad, c.RW2), np.float32)
    w2e[:c.C1, 0:c.C2] = W2
    w2e[:c.C1, c.EL2] = Wl2
    w2e[:c.C1, c.ER2] = Wr2
    w2e = w2e.reshape(c.KT2, 128, c.RW2).astype(BF)

    b1r = np.tile(b1[None, :], (128, 1)).astype(np.float32)
    b2r = np.tile(b2[None, :], (128, 1)).astype(np.float32)
    iot = np.tile(np.arange(128, dtype=np.float32)[None, :], (128, 1)).astype(BF)
    iop = np.arange(128, dtype=np.float32)[:, None].astype(BF)

    in_maps = []
    for r in range(c.NC):
        ft = np.zeros((c.Fpad, c.NBpad), np.float32)
        ft[:c.F, :c.NB] = feats[r * c.NB:(r + 1) * c.NB].T
        cd = core_data[r]
        in_maps.append(dict(
            featsT=ft.reshape(c.KT1, 128, c.NBpad).astype(BF),
            w1ext=w1e, w2ext=w2e, b1rep=b1r, b2rep=b2r, iota=iot, iotap=iop,
            g1lo=cd["g1lo"] if cd["g1lo"].shape[1] else
                np.zeros((128, 1), np.int16),
            g1hi=cd["g1hi"] if cd["g1hi"].shape[1] else
                np.zeros((128, 1), np.int16),
            dstloc=cd["dstloc"], dstlocT=cd["dstlocT"],
        ))
    return in_maps


_CACHE = {}


def _get_compiled(inputs, cfg):
    src = np.asarray(inputs["src"], np.int64)
    dst = np.asarray(inputs["dst"], np.int64)
    key = hashlib.sha1(np.ascontiguousarray(src).tobytes()
                       + np.ascontiguousarray(dst).tobytes()).hexdigest()
    if key not in _CACHE:
        plan, core_data = make_plan(src, dst, cfg)
        nc = build_program(cfg, plan)
        _CACHE[key] = (nc, plan, core_data)
    return _CACHE[key]


def kernel(**inputs) -> np.ndarray:
    feats = np.asarray(inputs["feats"])
    H, HID = np.asarray(inputs["attn_l1"]).shape
    cfg = Cfg(N=feats.shape[0], E=np.asarray(inputs["src"]).shape[0],
              F=feats.shape[1], HID=HID, HEADS=H)
    nc, plan, core_data = _get_compiled(inputs, cfg)
    in_maps = make_inputs(inputs, cfg, plan, core_data)
    res = bass_utils.run_bass_kernel_spmd(
        nc, in_maps, core_ids=list(range(cfg.NC)), trace=False)
    return np.concatenate([res.results[r]["out"] for r in range(cfg.NC)], axis=0)


# ---------------------------------------------------------------------------
# device-resident timing runner


class Runner:
    """Compiled SPMD executable with device-resident inputs."""

    def __init__(self, nc, in_maps, n_cores):
        import jax
        from jax.experimental.shard_map import shard_map
        from jax.sharding import Mesh, PartitionSpec
        from concourse import bass2jax, mybir as mb

        bass2jax.install_neuronx_cc_hook()
        pid_name = (nc.partition_id_tensor.name
                    if nc.partition_id_tensor else None)
        in_names, out_names, out_avals, zero_outs = [], [], [], []
        for alloc in nc.m.functions[0].allocations:
            if not isinstance(alloc, mb.MemoryLocationSet):
                continue
            name = alloc.memorylocations[0].name
            if alloc.kind == "ExternalInput":
                if name != pid_name:
                    in_names.append(name)
            elif alloc.kind == "ExternalOutput":
                out_names.append(name)
                out_avals.append(jax.core.ShapedArray(
                    tuple(alloc.tensor_shape), mb.dt.np(alloc.dtype)))
                zero_outs.append(np.zeros(alloc.tensor_shape,
                                          mb.dt.np(alloc.dtype)))
        n_params = len(in_names)
        all_names = in_names + out_names

        if pid_name is not None:
            all_names = all_names + [pid_name]

        def _body(*args):
            operands = list(args)
            if pid_name is not None:
                operands.append(bass2jax.partition_id_tensor())
            outs = bass2jax._bass_exec_p.bind(
                *operands, out_avals=tuple(out_avals), in_names=tuple(all_names),
                out_names=tuple(out_names), lowering_input_output_aliases=(),
                sim_require_finite=True, sim_require_nnan=True, nc=nc)
            return tuple(outs)

        devices = jax.devices()[:n_cores]
        mesh = Mesh(np.asarray(devices), ("core",))
        specs = (PartitionSpec("core"),) * (n_params + len(out_names))
        self._fn = jax.jit(shard_map(_body, mesh=mesh, in_specs=specs,
                                     out_specs=(PartitionSpec("core"),) * len(out_names),
                                     check_rep=False), keep_unused=True)
        concat_in = [np.concatenate([np.asarray(in_maps[c][nm])
                                     for c in range(n_cores)], axis=0)
                     for nm in in_names]
        concat_zero = [np.zeros((n_cores * z.shape[0], *z.shape[1:]), z.dtype)
                       for z in zero_outs]
        self._args = [jax.device_put(a) for a in concat_in + concat_zero]
        self.out_names, self.out_avals, self.n_cores = out_names, out_avals, n_cores

    def run(self):
        outs = self._fn(*self._args)
        for o in outs:
            o.block_until_ready()
        return outs

    def results(self):
        import numpy as _np
        outs = self.run()
        return [
            {nm: _np.asarray(outs[i]).reshape(self.n_cores,
                                              *self.out_avals[i].shape)[c]
             for i, nm in enumerate(self.out_names)}
            for c in range(self.n_cores)
        ]

    def time_ns(self, iters=12, warmup=3):
        for _ in range(warmup):
            self.run()
        best = float("inf")
        for _ in range(iters):
            t0 = time.perf_counter()
            self.run()
            best = min(best, time.perf_counter() - t0)
        return best * 1e9


def measure_hw_ns(inputs, reps_hi=9, phases="ABCEF", iters=12):
    """Device time per kernel via repeat-delta: (t[R] - t[1]) / (R - 1)."""
    feats = np.asarray(inputs["feats"])
    H, HID = np.asarray(inputs["attn_l1"]).shape
    cfg = Cfg(N=feats.shape[0], E=np.asarray(inputs["src"]).shape[0],
              F=feats.shape[1], HID=HID, HEADS=H)
    src = np.asarray(inputs["src"], np.int64)
    dst = np.asarray(inputs["dst"], np.int64)
    plan, core_data = make_plan(src, dst, cfg)
    in_maps = make_inputs(inputs, cfg, plan, core_data)
    t = {}
    for reps in (1, reps_hi):
        nc = build_program(cfg, plan, reps=reps, phases=phases)
        r = Runner(nc, in_maps, cfg.NC)
        t[reps] = r.time_ns(iters=iters)
        del r
    return (t[reps_hi] - t[1]) / (reps_hi - 1)



# revision 8
# speedup vs baseline: 1.5329x; 1.5329x over previous
"""GAT (2-layer, DGL-style) Bass kernel for Trainium2, 8-core SPMD. v3.

Strategy (dst-sharded, edge-parallel within core):
- Node rows sharded across 8 cores by dst range (6250/core).
- GEMM1 folds el/er attention dots into extended W columns; row = 640
  [msg 512 | el 8 | er 8 | pad]; AllGather -> replicated hfull.
- Edge phase per 128-dst-block: src rows fetched with dma_gather over 4
  SWDGE queues; gathers software-pipelined one group ahead of compute.
- msg columns stored (f,h)-major (host-permuted W1/b1/W2-rows) so the
  attention-weight multiply and normalize run in DVE 2x packed mode.
- ohT built by TensorE transpose of oh (no dstlocT input, no 2nd
  is_equal); PSUM->SBUF copies on the scalar engine to unload DVE.
- Segment softmax numerator/denominator accumulate per block with
  one-hot matmuls into PSUM (exp(e) written into the slab el slot so
  one rhs covers msg[256:512]+a).
- GEMM2 fused per block (TensorE transpose + matmul) -> bounce_h2.
- No segment-max subtraction (reference's emax cancels exactly).
"""

import sys
import time
import hashlib

for _p in ("/opt/trn_rl_repo", "/root/.axon_site", "/root/.axon_site/_ro/trn_rl_repo"):
    if _p not in sys.path:
        sys.path.insert(0, _p)

import numpy as np
import ml_dtypes

import concourse.bass as bass
import concourse.mybir as mybir
import concourse.tile as tile
from concourse import bacc
from concourse import bass_utils

F32 = mybir.dt.float32
BF16 = mybir.dt.bfloat16
I16 = mybir.dt.int16
BF = ml_dtypes.bfloat16

NEG_SLOPE = 0.2
DENOM_EPS = 1e-30
NQ = 4  # SWDGE queues for gather round-robin


# ---------------------------------------------------------------------------
# configuration


class Cfg:
    def __init__(self, N, E, F, HID, HEADS, NC=8, GBLK=1):
        self.N, self.E, self.F, self.HID, self.HEADS, self.NC = N, E, F, HID, HEADS, NC
        self.HALF = N // 2
        self.NB = N // NC                      # nodes per core
        self.NBLK = (self.NB + 127) // 128     # dst blocks per core
        self.NBpad = self.NBLK * 128
        self.GBLK = GBLK
        self.KT1 = (F + 127) // 128
        self.Fpad = self.KT1 * 128
        self.C1 = HEADS * HID                  # 512
        self.KT2 = (self.C1 + 127) // 128      # 4
        self.C1pad = self.KT2 * 128
        self.C2 = HID                          # 64
        # layer-1 row: [msg 512 | el 8 | er 8 | pad 112] -> 640 (1280 B)
        self.RW1 = 640
        self.EL1 = self.C1                     # el offset
        self.ER1 = self.C1 + HEADS             # er offset
        # layer-2 row: [msg 64 | el 1 | er 1 | pad] -> 128 (256 B)
        self.RW2 = 128
        self.EL2 = self.C2
        self.ER2 = self.C2 + 1
        assert self.HALF < 32768 and self.NB < 32768


# ---------------------------------------------------------------------------
# host-side edge preprocessing


def wrap_idxs(idx: np.ndarray) -> np.ndarray:
    """int16 idx list (len = nt*128) -> [128, nt*8] wrapped-16, replicated."""
    n = len(idx)
    assert n % 128 == 0
    m = n // 16
    out = np.zeros((16, m), dtype=np.int16)
    out[np.arange(n) % 16, np.arange(n) // 16] = idx
    return np.tile(out, (8, 1))


def make_plan(src, dst, cfg: Cfg):
    """Static tile plan (shared across cores) + per-core edge tensors."""
    NC, NB, NBLK, HALF, GBLK = cfg.NC, cfg.NB, cfg.NBLK, cfg.HALF, cfg.GBLK

    per_core = []
    counts = np.zeros((NC, NBLK, 2), dtype=np.int64)
    for r in range(NC):
        sel = np.nonzero((dst >= r * NB) & (dst < (r + 1) * NB))[0]
        es = src[sel].astype(np.int64)
        ed = (dst[sel] - r * NB).astype(np.int64)
        blk = ed >> 7
        half = (es >= HALF).astype(np.int64)
        key = blk * 2 + half
        o = np.argsort(key, kind="stable")
        es, ed, key = es[o], ed[o], key[o]
        cnt = np.bincount(key, minlength=NBLK * 2)
        counts[r] = cnt.reshape(NBLK, 2)
        per_core.append((es, ed, np.concatenate([[0], np.cumsum(cnt)])))

    cmax = counts.max(axis=0)                  # [NBLK, 2]
    ntile = (cmax + 127) // 128

    groups = []
    olo = ohi = oall = 0
    for g0 in range(0, NBLK, GBLK):
        blocks = list(range(g0, min(g0 + GBLK, NBLK)))
        ntlo = int(sum(ntile[b, 0] for b in blocks))
        nthi = int(sum(ntile[b, 1] for b in blocks))
        nt = ntlo + nthi
        tile_blk = []
        for b in blocks:
            tile_blk += [b] * int(ntile[b, 0])
        for b in blocks:
            tile_blk += [b] * int(ntile[b, 1])
        groups.append(dict(blocks=blocks, ntlo=ntlo, nthi=nthi, nt=nt,
                           tile_blk=tile_blk, olo=olo, ohi=ohi, oall=oall))
        olo, ohi, oall = olo + ntlo, ohi + nthi, oall + nt

    # per-core flat arrays in group/tile order
    core_data = []
    for r in range(NC):
        es, ed, cum = per_core[r]
        lo_idx, hi_idx, dvals = [], [], []
        for g in groups:
            for h, acc in ((0, lo_idx), (1, hi_idx)):
                for b in g["blocks"]:
                    k = b * 2 + h
                    s, e = cum[k], cum[k + 1]
                    n_pad = int(ntile[b, h]) * 128
                    v = es[s:e] - (HALF if h else 0)
                    v = np.concatenate([v, np.zeros(n_pad - len(v), np.int64)])
                    acc.append(v)
                    d = np.concatenate([ed[s:e],
                                        np.full(n_pad - (e - s), -1, np.int64)])
                    dv = np.where(d >= 0, d & 127, 999).astype(np.float32)
                    dvals.append(dv)
        lo = np.concatenate(lo_idx) if lo_idx else np.zeros(0, np.int64)
        hi = np.concatenate(hi_idx) if hi_idx else np.zeros(0, np.int64)
        dv = np.concatenate(dvals)
        # wrap per group; dl in [128, nt] layout
        glo, ghi, gdl = [], [], []
        plo = phi = pall = 0
        for g in groups:
            nlo, nhi, nt = g["ntlo"] * 128, g["nthi"] * 128, g["nt"] * 128
            glo.append(wrap_idxs(lo[plo:plo + nlo].astype(np.int16)))
            ghi.append(wrap_idxs(hi[phi:phi + nhi].astype(np.int16)))
            dvg = dv[pall:pall + nt]
            gdl.append(dvg.reshape(g["nt"], 128).T)
            plo, phi, pall = plo + nlo, phi + nhi, pall + nt
        core_data.append(dict(
            g1lo=np.concatenate(glo, axis=1) if glo else np.zeros((128, 0), np.int16),
            g1hi=np.concatenate(ghi, axis=1) if ghi else np.zeros((128, 0), np.int16),
            dstloc=np.concatenate(gdl, axis=1).astype(BF),
        ))

    plan = dict(groups=groups, ntile=ntile,
                TLO=int(sum(g["ntlo"] for g in groups)),
                THI=int(sum(g["nthi"] for g in groups)),
                TT=int(sum(g["nt"] for g in groups)))
    return plan, core_data


MAX_GIDX = 1024  # HW limit: dma_gather crashes above 1024 indices

_gq = [0]


def chunked_gather(nc, out_slab, in_ap, idx_tile, t0, ntc, elem, step=None,
                   nq=1):
    CT = MAX_GIDX // 128
    for q0 in range(0, ntc, CT):
        qn = min(CT, ntc - q0)
        kw = dict(elem_step=step) if step else {}
        nc.gpsimd.dma_gather(
            out_ap=out_slab[:, t0 + q0:t0 + q0 + qn, :], in_ap=in_ap,
            idxs_ap=idx_tile[:, q0 * 8:(q0 + qn) * 8],
            num_idxs=qn * 128, num_idxs_reg=qn * 128, elem_size=elem,
            queue_num=_gq[0] % nq, **kw)
        _gq[0] += 1


# ---------------------------------------------------------------------------
# program builder


def build_program(cfg: Cfg, plan, reps=1, phases="ABCEF", force_shared=False,
                  nq=NQ):
    ag_space = "Shared" if (reps == 1 or force_shared) else "Local"
    c = cfg
    H = c.HEADS
    nc = bacc.Bacc("TRN2", target_bir_lowering=False, debug=False,
                   num_devices=c.NC, num_swdge_queues=nq)
    _gq[0] = 0

    dt = nc.dram_tensor
    featsT = dt("featsT", [c.KT1, 128, c.NBpad], BF16, kind="ExternalInput")
    w1ext = dt("w1ext", [c.KT1, 128, c.RW1], BF16, kind="ExternalInput")
    w2ext = dt("w2ext", [c.KT2, 128, c.RW2], BF16, kind="ExternalInput")
    b1rep = dt("b1rep", [128, c.C1], BF16, kind="ExternalInput")
    b2rep = dt("b2rep", [128, c.C2], F32, kind="ExternalInput")
    iota = dt("iota", [128, 128], BF16, kind="ExternalInput")
    iotap = dt("iotap", [128, 1], BF16, kind="ExternalInput")
    g1lo = dt("g1lo", [128, max(1, plan["TLO"] * 8)], I16, kind="ExternalInput")
    g1hi = dt("g1hi", [128, max(1, plan["THI"] * 8)], I16, kind="ExternalInput")
    dstloc = dt("dstloc", [128, plan["TT"]], BF16, kind="ExternalInput")
    out = dt("out", [c.NB, c.C2], F32, kind="ExternalOutput")

    groups = plan["groups"]

    with tile.TileContext(nc) as tc:
        with tc.tile_pool(name="dram", bufs=1, space="DRAM") as dram:
            bounce_h = dram.tile([c.NB, c.RW1], BF16)
            bounce_h2 = dram.tile([c.NB, c.RW2], BF16)

            for _rep in range(reps):
                # Shared tiles allow a single writer only -> per-rep tiles.
                hfull = dram.tile([c.N, c.RW1], BF16, addr_space=ag_space)
                h2full = dram.tile([c.N, c.RW2], BF16, addr_space=ag_space)

                if "A" in phases:
                    # ------------- phase A: GEMM1 -> bounce_h -------------
                    with (
                        nc.named_scope("phaseA_gemm1"),
                        tc.tile_pool(name="ga", bufs=1) as cpool,
                        tc.tile_pool(name="gaw", bufs=2) as wpool,
                        tc.tile_pool(name="gap", bufs=2, space="PSUM") as pspool,
                    ):
                        w1sb = cpool.tile([128, c.KT1, c.RW1], BF16)
                        ftsb = cpool.tile([128, c.KT1, c.NBpad], BF16)
                        nc.sync.dma_start(w1sb[:], w1ext[:].rearrange("k p w -> p k w"))
                        nc.sync.dma_start(ftsb[:], featsT[:].rearrange("k p w -> p k w"))
                        for ntb in range(c.NBLK):
                            pa = pspool.tile([128, 512], F32, tag="pa", space="PSUM")
                            pb = pspool.tile([128, c.RW1 - 512], F32, tag="pb",
                                             space="PSUM")
                            for k in range(c.KT1):
                                lhsT = ftsb[:, k, ntb * 128:(ntb + 1) * 128]
                                nc.tensor.matmul(pa[:], lhsT, w1sb[:, k, 0:512],
                                                 start=(k == 0), stop=(k == c.KT1 - 1))
                                nc.tensor.matmul(pb[:], lhsT,
                                                 w1sb[:, k, 512:c.RW1],
                                                 start=(k == 0), stop=(k == c.KT1 - 1))
                            ht = wpool.tile([128, c.RW1], BF16, tag="ht")
                            nc.scalar.copy(ht[:, 0:512], pa[:])
                            nc.scalar.copy(ht[:, 512:c.RW1], pb[:])
                            rows = min(128, c.NB - ntb * 128)
                            nc.sync.dma_start(bounce_h[ntb * 128:ntb * 128 + rows, :],
                                              ht[:rows, :])

                if "B" in phases:
                    # ------------- phase B: AllGather h -------------
                    with nc.named_scope("phaseB_ag1"):
                        nc.gpsimd.collective_compute(
                            "AllGather", mybir.AluOpType.bypass,
                            replica_groups=[list(range(c.NC))],
                            ins=[bounce_h.opt()], outs=[hfull.opt()],
                        )

                if "g" in phases:
                    # ---- phase g: layer-1 gathers ONLY (timing isolation) ----
                    with (
                        nc.named_scope("phaseG_gatheronly"),
                        tc.tile_pool(name="g1", bufs=3) as sb,
                    ):
                        for g in groups:
                            ntlo, nthi, nt = g["ntlo"], g["nthi"], g["nt"]
                            if nt == 0:
                                continue
                            slab = sb.tile([128, nt, c.RW1], BF16, tag="slab")
                            if ntlo:
                                ilo = sb.tile([128, ntlo * 8], I16, tag="ilo")
                                nc.sync.dma_start(
                                    ilo[:], g1lo[:, g["olo"] * 8:(g["olo"] + ntlo) * 8])
                                chunked_gather(nc, slab, hfull[0:c.HALF, :], ilo,
                                               0, ntlo, c.RW1, nq=nq)
                            if nthi:
                                ihi = sb.tile([128, nthi * 8], I16, tag="ihi")
                                nc.sync.dma_start(
                                    ihi[:], g1hi[:, g["ohi"] * 8:(g["ohi"] + nthi) * 8])
                                chunked_gather(nc, slab, hfull[c.HALF:c.N, :], ihi,
                                               ntlo, nthi, c.RW1, nq=nq)

                if "C" in phases or "c" in phases:
                    # ------- phase C: layer-1 edge phase + fused GEMM2 -------
                    # "c" variant: sequential slab fill to time compute side
                    do_gather = "C" in phases
                    with (
                        nc.named_scope("phaseC_edge1"),
                        tc.tile_pool(name="ec", bufs=1) as cst,
                        tc.tile_pool(name="e1g", bufs=3) as sbg,
                        tc.tile_pool(name="e1", bufs=2) as sb,
                        tc.tile_pool(name="e1n", bufs=2) as nsb,
                        tc.tile_pool(name="e1p", bufs=2, space="PSUM") as ps,
                        tc.tile_pool(name="e1e", bufs=1, space="PSUM") as pse,
                        tc.tile_pool(name="e1q", bufs=1, space="PSUM") as psq,
                        tc.tile_pool(name="e1t", bufs=2, space="PSUM") as pst,
                    ):
                        iosb = cst.tile([128, 128], BF16)
                        nc.sync.dma_start(iosb[:], iota[:])
                        iopsb = cst.tile([128, 1], BF16)
                        nc.sync.dma_start(iopsb[:], iotap[:])
                        b1sb = cst.tile([128, c.C1], BF16)
                        nc.sync.dma_start(b1sb[:], b1rep[:])
                        ident = cst.tile([128, 128], BF16)
                        nc.vector.memset(ident[:], 0.0)
                        nc.vector.tensor_tensor(
                            out=ident[:], in0=iosb[:],
                            in1=iopsb[:].broadcast_to([128, 128]),
                            op=mybir.AluOpType.is_equal)
                        w2sb = cst.tile([128, c.KT2, c.RW2], BF16)
                        nc.sync.dma_start(w2sb[:], w2ext[:].rearrange("k p w -> p k w"))
                        # relu(b1) -> xb0; its fused-GEMM2 row (for empty blocks)
                        xb0 = cst.tile([128, c.C1], BF16)
                        nc.scalar.activation(xb0[:], b1sb[:],
                                             mybir.ActivationFunctionType.Relu)
                        xt20 = cst.tile([128, c.KT2, 128], BF16)
                        h2t0 = cst.tile([128, c.RW2], BF16)
                        pt0 = pst.tile([128, 128], BF16, tag="pt", space="PSUM")
                        for k in range(c.KT2):
                            nc.tensor.transpose(
                                pt0[:], xb0[:, k * 128:(k + 1) * 128], ident[:])
                            nc.scalar.copy(xt20[:, k, :], pt0[:])
                        pc20 = psq.tile([128, c.RW2], F32, tag="pc2", space="PSUM")
                        for k in range(c.KT2):
                            nc.tensor.matmul(pc20[:], xt20[:, k, :], w2sb[:, k, :],
                                             start=(k == 0), stop=(k == c.KT2 - 1))
                        nc.scalar.copy(h2t0[:], pc20[:])

                        def c_loads(g):
                            ntlo, nthi, nt = g["ntlo"], g["nthi"], g["nt"]
                            slab = sbg.tile([128, nt, c.RW1], BF16, tag="slab")
                            dl = sbg.tile([128, nt], BF16, tag="dl")
                            erb = sbg.tile([128, c.GBLK, H], BF16, tag="erb")
                            nc.sync.dma_start(
                                dl[:], dstloc[:, g["oall"]:g["oall"] + nt])
                            nc.vector.memset(erb[:], 0.0)
                            for j, b in enumerate(g["blocks"]):
                                rows = min(128, c.NB - b * 128)
                                nc.sync.dma_start(
                                    erb[:rows, j, :],
                                    bounce_h[b * 128:b * 128 + rows,
                                             c.ER1:c.ER1 + H])
                            if do_gather:
                                if ntlo:
                                    ilo = sbg.tile([128, ntlo * 8], I16, tag="ilo")
                                    nc.sync.dma_start(
                                        ilo[:],
                                        g1lo[:, g["olo"] * 8:(g["olo"] + ntlo) * 8])
                                    chunked_gather(nc, slab, hfull[0:c.HALF, :],
                                                   ilo, 0, ntlo, c.RW1, nq=nq)
                                if nthi:
                                    ihi = sbg.tile([128, nthi * 8], I16, tag="ihi")
                                    nc.sync.dma_start(
                                        ihi[:],
                                        g1hi[:, g["ohi"] * 8:(g["ohi"] + nthi) * 8])
                                    chunked_gather(nc, slab, hfull[c.HALF:c.N, :],
                                                   ihi, ntlo, nthi, c.RW1, nq=nq)
                            else:
                                nc.sync.dma_start(
                                    slab[:],
                                    hfull[0:nt * 128, :].rearrange(
                                        "(t p) e -> p t e", p=128))
                            return slab, dl, erb

                        def c_compute(g, slab, dl, erb):
                            nt = g["nt"]
                            # one-hot [e, d]; ohT via TensorE transpose
                            oh = sb.tile([128, nt, 128], BF16, tag="oh")
                            ohT = sb.tile([128, nt, 128], BF16, tag="ohT")
                            nc.vector.tensor_tensor(
                                out=oh[:],
                                in0=iosb[:, None, :].broadcast_to([128, nt, 128]),
                                in1=dl[:, :, None].broadcast_to([128, nt, 128]),
                                op=mybir.AluOpType.is_equal)
                            erg = pse.tile([128, nt, H], F32, tag="erg",
                                           space="PSUM")
                            for t, tb in enumerate(g["tile_blk"]):
                                j = g["blocks"].index(tb)
                                ptT = pst.tile([128, 128], BF16, tag="pt",
                                               space="PSUM")
                                nc.tensor.transpose(ptT[:], oh[:, t, :], ident[:])
                                nc.scalar.copy(ohT[:, t, :], ptT[:])
                                nc.tensor.matmul(erg[:, t, :], ohT[:, t, :],
                                                 erb[:, j, :],
                                                 start=True, stop=True)
                            # e = el + er ; leaky-relu ; exp -> a (el slot)
                            et = sb.tile([128, nt, H], F32, tag="et")
                            e2 = sb.tile([128, nt, H], F32, tag="e2")
                            nc.vector.tensor_tensor(
                                out=et[:], in0=slab[:, :, c.EL1:c.EL1 + H],
                                in1=erg[:], op=mybir.AluOpType.add)
                            nc.vector.scalar_tensor_tensor(
                                out=e2[:], in0=et[:], scalar=NEG_SLOPE,
                                in1=et[:], op0=mybir.AluOpType.mult,
                                op1=mybir.AluOpType.max)
                            nc.scalar.activation(slab[:, :, c.EL1:c.EL1 + H],
                                                 e2[:],
                                                 mybir.ActivationFunctionType.Exp)
                            # msg = h * a ((f,h)-major: contiguous 8-runs)
                            nc.vector.tensor_tensor(
                                out=slab[:, :, 0:c.C1].rearrange(
                                    "p t (f h) -> p t f h", h=H),
                                in0=slab[:, :, 0:c.C1].rearrange(
                                    "p t (f h) -> p t f h", h=H),
                                in1=slab[:, :, c.EL1:c.EL1 + H][:, :, None, :]
                                .broadcast_to([128, nt, c.HID, H]),
                                op=mybir.AluOpType.mult)
                            # per-block accumulate + normalize + fused GEMM2
                            for b in g["blocks"]:
                                tlist = [t for t, tb in enumerate(g["tile_blk"])
                                         if tb == b]
                                rows = min(128, c.NB - b * 128)
                                if not tlist:
                                    nc.sync.dma_start(
                                        bounce_h2[b * 128:b * 128 + rows, :],
                                        h2t0[:rows, :])
                                    continue
                                pa = ps.tile([128, 256], F32, tag="pa",
                                             space="PSUM")
                                pb = ps.tile([128, 264], F32, tag="pb",
                                             space="PSUM")
                                for j, t in enumerate(tlist):
                                    st, sp = (j == 0), (j == len(tlist) - 1)
                                    nc.tensor.matmul(pa[:], oh[:, t, :],
                                                     slab[:, t, 0:256],
                                                     start=st, stop=sp)
                                    nc.tensor.matmul(pb[:], oh[:, t, :],
                                                     slab[:, t, 256:520],
                                                     start=st, stop=sp)
                                dg = nsb.tile([128, H], F32, tag="dg")
                                rd = nsb.tile([128, H], F32, tag="rd")
                                nc.vector.tensor_scalar_max(dg[:], pb[:, 256:264],
                                                            DENOM_EPS)
                                nc.vector.reciprocal(rd[:], dg[:])
                                # normalize ((f,h)-major) + bias -> bf16
                                xt = nsb.tile([128, c.C1], BF16, tag="xt")
                                F2 = 256 // H  # f-cols per half
                                nc.vector.tensor_tensor(
                                    out=xt[:, 0:256].rearrange(
                                        "p (f h) -> p f h", h=H),
                                    in0=pa[:].rearrange("p (f h) -> p f h", h=H),
                                    in1=rd[:, None, :].broadcast_to([128, F2, H]),
                                    op=mybir.AluOpType.mult)
                                nc.vector.tensor_tensor(
                                    out=xt[:, 256:512].rearrange(
                                        "p (f h) -> p f h", h=H),
                                    in0=pb[:, 0:256].rearrange(
                                        "p (f h) -> p f h", h=H),
                                    in1=rd[:, None, :].broadcast_to([128, F2, H]),
                                    op=mybir.AluOpType.mult)
                                nc.vector.tensor_tensor(out=xt[:], in0=xt[:],
                                                        in1=b1sb[:],
                                                        op=mybir.AluOpType.add)
                                xb = nsb.tile([128, c.C1], BF16, tag="xb")
                                nc.scalar.activation(xb[:], xt[:],
                                                     mybir.ActivationFunctionType.Relu)
                                # fused GEMM2 for this block
                                xt2 = nsb.tile([128, c.KT2, 128], BF16, tag="xt2")
                                for k in range(c.KT2):
                                    pt = pst.tile([128, 128], BF16, tag="pt",
                                                  space="PSUM")
                                    nc.tensor.transpose(
                                        pt[:], xb[:, k * 128:(k + 1) * 128],
                                        ident[:])
                                    nc.scalar.copy(xt2[:, k, :], pt[:])
                                pc2 = psq.tile([128, c.RW2], F32, tag="pc2",
                                               space="PSUM")
                                for k in range(c.KT2):
                                    nc.tensor.matmul(pc2[:], xt2[:, k, :],
                                                     w2sb[:, k, :],
                                                     start=(k == 0),
                                                     stop=(k == c.KT2 - 1))
                                h2t = nsb.tile([128, c.RW2], BF16, tag="h2t")
                                nc.scalar.copy(h2t[:], pc2[:])
                                nc.sync.dma_start(
                                    bounce_h2[b * 128:b * 128 + rows, :],
                                    h2t[:rows, :])

                        pend = None
                        for g in groups:
                            if g["nt"] == 0:
                                for b in g["blocks"]:
                                    rows = min(128, c.NB - b * 128)
                                    nc.sync.dma_start(
                                        bounce_h2[b * 128:b * 128 + rows, :],
                                        h2t0[:rows, :])
                                continue
                            tls = c_loads(g)
                            if pend is not None:
                                c_compute(pend[0], *pend[1])
                            pend = (g, tls)
                        if pend is not None:
                            c_compute(pend[0], *pend[1])

                if "E" in phases:
                    # ------------- phase E: AllGather h2 -------------
                    with nc.named_scope("phaseE_ag2"):
                        nc.gpsimd.collective_compute(
                            "AllGather", mybir.AluOpType.bypass,
                            replica_groups=[list(range(c.NC))],
                            ins=[bounce_h2.opt()], outs=[h2full.opt()],
                        )

                if "F" in phases:
                    # ------------- phase F: layer-2 edge phase -------------
                    with (
                        nc.named_scope("phaseF_edge2"),
                        tc.tile_pool(name="fc", bufs=1) as cst,
                        tc.tile_pool(name="f1g", bufs=3) as sbg,
                        tc.tile_pool(name="f1", bufs=2) as sb,
                        tc.tile_pool(name="f1n", bufs=2) as nsb,
                        tc.tile_pool(name="f1p", bufs=2, space="PSUM") as ps,
                        tc.tile_pool(name="f1t", bufs=2, space="PSUM") as pst,
                    ):
                        iosb = cst.tile([128, 128], BF16)
                        nc.sync.dma_start(iosb[:], iota[:])
                        iopsb = cst.tile([128, 1], BF16)
                        nc.sync.dma_start(iopsb[:], iotap[:])
                        b2sb = cst.tile([128, c.C2], F32)
                        nc.sync.dma_start(b2sb[:], b2rep[:])
                        ident = cst.tile([128, 128], BF16)
                        nc.vector.memset(ident[:], 0.0)
                        nc.vector.tensor_tensor(
                            out=ident[:], in0=iosb[:],
                            in1=iopsb[:].broadcast_to([128, 128]),
                            op=mybir.AluOpType.is_equal)
                        ot0 = cst.tile([128, c.C2], F32)
                        nc.vector.tensor_copy(ot0[:], b2sb[:])

                        def f_loads(g):
                            ntlo, nthi, nt = g["ntlo"], g["nthi"], g["nt"]
                            slab = sbg.tile([128, nt, c.RW2], BF16, tag="slab2")
                            dl = sbg.tile([128, nt], BF16, tag="dl2")
                            erb = sbg.tile([128, c.GBLK, 1], BF16, tag="erb2")
                            nc.sync.dma_start(
                                dl[:], dstloc[:, g["oall"]:g["oall"] + nt])
                            nc.vector.memset(erb[:], 0.0)
                            for j, b in enumerate(g["blocks"]):
                                rows = min(128, c.NB - b * 128)
                                nc.sync.dma_start(
                                    erb[:rows, j, :],
                                    bounce_h2[b * 128:b * 128 + rows,
                                              c.ER2:c.ER2 + 1])
                            if ntlo:
                                ilo = sbg.tile([128, ntlo * 8], I16, tag="ilo2")
                                nc.sync.dma_start(
                                    ilo[:], g1lo[:, g["olo"] * 8:(g["olo"] + ntlo) * 8])
                                chunked_gather(nc, slab, h2full[0:c.HALF, :],
                                               ilo, 0, ntlo, c.RW2, nq=nq)
                            if nthi:
                                ihi = sbg.tile([128, nthi * 8], I16, tag="ihi2")
                                nc.sync.dma_start(
                                    ihi[:], g1hi[:, g["ohi"] * 8:(g["ohi"] + nthi) * 8])
                                chunked_gather(nc, slab, h2full[c.HALF:c.N, :],
                                               ihi, ntlo, nthi, c.RW2, nq=nq)
                            return slab, dl, erb

                        def f_compute(g, slab, dl, erb):
                            nt = g["nt"]
                            oh = sb.tile([128, nt, 128], BF16, tag="oh2")
                            ohT = sb.tile([128, nt, 128], BF16, tag="ohT2")
                            nc.vector.tensor_tensor(
                                out=oh[:],
                                in0=iosb[:, None, :].broadcast_to([128, nt, 128]),
                                in1=dl[:, :, None].broadcast_to([128, nt, 128]),
                                op=mybir.AluOpType.is_equal)
                            erg = ps.tile([128, nt, 1], F32, tag="erg2",
                                          space="PSUM")
                            for t, tb in enumerate(g["tile_blk"]):
                                j = g["blocks"].index(tb)
                                ptT = pst.tile([128, 128], BF16, tag="ptT2",
                                               space="PSUM")
                                nc.tensor.transpose(ptT[:], oh[:, t, :], ident[:])
                                nc.scalar.copy(ohT[:, t, :], ptT[:])
                                nc.tensor.matmul(erg[:, t, :], ohT[:, t, :],
                                                 erb[:, j, :],
                                                 start=True, stop=True)
                            et = sb.tile([128, nt, 1], F32, tag="et2")
                            e2 = sb.tile([128, nt, 1], F32, tag="e22")
                            nc.vector.tensor_tensor(
                                out=et[:], in0=slab[:, :, c.EL2:c.EL2 + 1],
                                in1=erg[:], op=mybir.AluOpType.add)
                            nc.vector.scalar_tensor_tensor(
                                out=e2[:], in0=et[:], scalar=NEG_SLOPE,
                                in1=et[:], op0=mybir.AluOpType.mult,
                                op1=mybir.AluOpType.max)
                            nc.scalar.activation(slab[:, :, c.EL2:c.EL2 + 1],
                                                 e2[:],
                                                 mybir.ActivationFunctionType.Exp)
                            nc.vector.tensor_tensor(
                                out=slab[:, :, 0:c.C2],
                                in0=slab[:, :, 0:c.C2],
                                in1=slab[:, :, c.EL2:c.EL2 + 1].broadcast_to(
                                    [128, nt, c.C2]),
                                op=mybir.AluOpType.mult)
                            for b in g["blocks"]:
                                tlist = [t for t, tb in enumerate(g["tile_blk"])
                                         if tb == b]
                                rows = min(128, c.NB - b * 128)
                                if not tlist:
                                    nc.sync.dma_start(out[b * 128:b * 128 + rows, :],
                                                      ot0[:rows, :])
                                    continue
                                pc = ps.tile([128, c.C2 + 1], F32, tag="pc",
                                             space="PSUM")
                                for j, t in enumerate(tlist):
                                    nc.tensor.matmul(pc[:], oh[:, t, :],
                                                     slab[:, t, 0:c.C2 + 1],
                                                     start=(j == 0),
                                                     stop=(j == len(tlist) - 1))
                                dg = nsb.tile([128, 1], F32, tag="dg2")
                                rd = nsb.tile([128, 1], F32, tag="rd2")
                                nc.vector.tensor_scalar_max(dg[:],
                                                            pc[:, c.C2:c.C2 + 1],
                                                            DENOM_EPS)
                                nc.vector.reciprocal(rd[:], dg[:])
                                ot = nsb.tile([128, c.C2], F32, tag="ot")
                                nc.vector.tensor_scalar(
                                    out=ot[:], in0=pc[:, 0:c.C2],
                                    scalar1=rd[:, 0:1],
                                    scalar2=None, op0=mybir.AluOpType.mult)
                                nc.vector.tensor_tensor(out=ot[:], in0=ot[:],
                                                        in1=b2sb[:],
                                                        op=mybir.AluOpType.add)
                                nc.sync.dma_start(out[b * 128:b * 128 + rows, :],
                                                  ot[:rows, :])

                        pend = None
                        for g in groups:
                            if g["nt"] == 0:
                                for b in g["blocks"]:
                                    rows = min(128, c.NB - b * 128)
                                    nc.sync.dma_start(
                                        out[b * 128:b * 128 + rows, :],
                                        ot0[:rows, :])
                                continue
                            tls = f_loads(g)
                            if pend is not None:
                                f_compute(pend[0], *pend[1])
                            pend = (g, tls)
                        if pend is not None:
                            f_compute(pend[0], *pend[1])

    nc.compile()
    return nc


# ---------------------------------------------------------------------------
# host orchestration


def make_inputs(inputs, cfg: Cfg, plan, core_data):
    c = cfg
    feats = np.asarray(inputs["feats"], np.float32)
    W1 = np.asarray(inputs["W1"], np.float32)
    al1 = np.asarray(inputs["attn_l1"], np.float32)
    ar1 = np.asarray(inputs["attn_r1"], np.float32)
    b1 = np.asarray(inputs["b1"], np.float32)
    W2 = np.asarray(inputs["W2"], np.float32)
    al2 = np.asarray(inputs["attn_l2"], np.float32)
    ar2 = np.asarray(inputs["attn_r2"], np.float32)
    b2 = np.asarray(inputs["b2"], np.float32)

    H, HID = c.HEADS, c.HID
    # (f,h)-major permutation: new col j = f*H + h <- old col h*HID + f
    jj = np.arange(c.C1)
    perm = (jj % H) * HID + (jj // H)

    W1r = W1.reshape(c.F, H, HID)
    Wl1 = np.einsum("khd,hd->kh", W1r, al1)
    Wr1 = np.einsum("khd,hd->kh", W1r, ar1)
    w1e = np.zeros((c.Fpad, c.RW1), np.float32)
    w1e[:c.F, 0:c.C1] = W1[:, perm]
    w1e[:c.F, c.EL1:c.EL1 + H] = Wl1
    w1e[:c.F, c.ER1:c.ER1 + H] = Wr1
    w1e = w1e.reshape(c.KT1, 128, c.RW1).astype(BF)

    Wl2 = W2 @ al2[0]
    Wr2 = W2 @ ar2[0]
    w2e = np.zeros((c.C1pad, c.RW2), np.float32)
    w2e[:c.C1, 0:c.C2] = W2[perm, :]
    w2e[:c.C1, c.EL2] = Wl2[perm]
    w2e[:c.C1, c.ER2] = Wr2[perm]
    w2e = w2e.reshape(c.KT2, 128, c.RW2).astype(BF)

    b1r = np.tile(b1[perm][None, :], (128, 1)).astype(BF)
    b2r = np.tile(b2[None, :], (128, 1)).astype(np.float32)
    iot = np.tile(np.arange(128, dtype=np.float32)[None, :], (128, 1)).astype(BF)
    iop = np.arange(128, dtype=np.float32)[:, None].astype(BF)

    in_maps = []
    for r in range(c.NC):
        ft = np.zeros((c.Fpad, c.NBpad), np.float32)
        ft[:c.F, :c.NB] = feats[r * c.NB:(r + 1) * c.NB].T
        cd = core_data[r]
        in_maps.append(dict(
            featsT=ft.reshape(c.KT1, 128, c.NBpad).astype(BF),
            w1ext=w1e, w2ext=w2e, b1rep=b1r, b2rep=b2r, iota=iot, iotap=iop,
            g1lo=cd["g1lo"] if cd["g1lo"].shape[1] else
                np.zeros((128, 1), np.int16),
            g1hi=cd["g1hi"] if cd["g1hi"].shape[1] else
                np.zeros((128, 1), np.int16),
            dstloc=cd["dstloc"],
        ))
    return in_maps


_CACHE = {}


def _get_compiled(inputs, cfg):
    src = np.asarray(inputs["src"], np.int64)
    dst = np.asarray(inputs["dst"], np.int64)
    key = hashlib.sha1(np.ascontiguousarray(src).tobytes()
                       + np.ascontiguousarray(dst).tobytes()).hexdigest()
    if key not in _CACHE:
        plan, core_data = make_plan(src, dst, cfg)
        nc = build_program(cfg, plan)
        _CACHE[key] = (nc, plan, core_data)
    return _CACHE[key]


def kernel(**inputs) -> np.ndarray:
    feats = np.asarray(inputs["feats"])
    H, HID = np.asarray(inputs["attn_l1"]).shape
    cfg = Cfg(N=feats.shape[0], E=np.asarray(inputs["src"]).shape[0],
              F=feats.shape[1], HID=HID, HEADS=H)
    nc, plan, core_data = _get_compiled(inputs, cfg)
    in_maps = make_inputs(inputs, cfg, plan, core_data)
    res = bass_utils.run_bass_kernel_spmd(
        nc, in_maps, core_ids=list(range(cfg.NC)), trace=False)
    return np.concatenate([res.results[r]["out"] for r in range(cfg.NC)], axis=0)


# ---------------------------------------------------------------------------
# device-resident timing runner


class Runner:
    """Compiled SPMD executable with device-resident inputs."""

    def __init__(self, nc, in_maps, n_cores):
        import jax
        from jax.experimental.shard_map import shard_map
        from jax.sharding import Mesh, PartitionSpec
        from concourse import bass2jax, mybir as mb

        bass2jax.install_neuronx_cc_hook()
        pid_name = (nc.partition_id_tensor.name
                    if nc.partition_id_tensor else None)
        in_names, out_names, out_avals, zero_outs = [], [], [], []
        for alloc in nc.m.functions[0].allocations:
            if not isinstance(alloc, mb.MemoryLocationSet):
                continue
            name = alloc.memorylocations[0].name
            if alloc.kind == "ExternalInput":
                if name != pid_name:
                    in_names.append(name)
            elif alloc.kind == "ExternalOutput":
                out_names.append(name)
                out_avals.append(jax.core.ShapedArray(
                    tuple(alloc.tensor_shape), mb.dt.np(alloc.dtype)))
                zero_outs.append(np.zeros(alloc.tensor_shape,
                                          mb.dt.np(alloc.dtype)))
        n_params = len(in_names)
        all_names = in_names + out_names

        if pid_name is not None:
            all_names = all_names + [pid_name]

        def _body(*args):
            operands = list(args)
            if pid_name is not None:
                operands.append(bass2jax.partition_id_tensor())
            outs = bass2jax._bass_exec_p.bind(
                *operands, out_avals=tuple(out_avals), in_names=tuple(all_names),
                out_names=tuple(out_names), lowering_input_output_aliases=(),
                sim_require_finite=True, sim_require_nnan=True, nc=nc)
            return tuple(outs)

        devices = jax.devices()[:n_cores]
        mesh = Mesh(np.asarray(devices), ("core",))
        specs = (PartitionSpec("core"),) * (n_params + len(out_names))
        self._fn = jax.jit(shard_map(_body, mesh=mesh, in_specs=specs,
                                     out_specs=(PartitionSpec("core"),) * len(out_names),
                                     check_rep=False), keep_unused=True)
        concat_in = [np.concatenate([np.asarray(in_maps[c][nm])
                                     for c in range(n_cores)], axis=0)
                     for nm in in_names]
        concat_zero = [np.zeros((n_cores * z.shape[0], *z.shape[1:]), z.dtype)
                       for z in zero_outs]
        self._args = [jax.device_put(a) for a in concat_in + concat_zero]
        self.out_names, self.out_avals, self.n_cores = out_names, out_avals, n_cores

    def run(self):
        outs = self._fn(*self._args)
        for o in outs:
            o.block_until_ready()
        return outs

    def results(self):
        import numpy as _np
        outs = self.run()
        return [
            {nm: _np.asarray(outs[i]).reshape(self.n_cores,
                                              *self.out_avals[i].shape)[c]
             for i, nm in enumerate(self.out_names)}
            for c in range(self.n_cores)
        ]

    def time_ns(self, iters=12, warmup=3):
        for _ in range(warmup):
            self.run()
        best = float("inf")
        for _ in range(iters):
            t0 = time.perf_counter()
            self.run()
            best = min(best, time.perf_counter() - t0)
        return best * 1e9


def measure_hw_ns(inputs, reps_hi=9, phases="ABCEF", iters=12):
    """Device time per kernel via repeat-delta: (t[R] - t[1]) / (R - 1)."""
    feats = np.asarray(inputs["feats"])
    H, HID = np.asarray(inputs["attn_l1"]).shape
    cfg = Cfg(N=feats.shape[0], E=np.asarray(inputs["src"]).shape[0],
              F=feats.shape[1], HID=HID, HEADS=H)
    src = np.asarray(inputs["src"], np.int64)
    dst = np.asarray(inputs["dst"], np.int64)
    plan, core_data = make_plan(src, dst, cfg)
    in_maps = make_inputs(inputs, cfg, plan, core_data)
    t = {}
    for reps in (1, reps_hi):
        nc = build_program(cfg, plan, reps=reps, phases=phases,
                           force_shared=True)
        r = Runner(nc, in_maps, cfg.NC)
        t[reps] = r.time_ns(iters=iters)
        del r
    return (t[reps_hi] - t[1]) / (reps_hi - 1)


# revision 13
# speedup vs baseline: 1.6507x; 1.0769x over previous
"""GAT (2-layer, DGL-style) Bass kernel for Trainium2, 8-core SPMD. v3.

Strategy (dst-sharded, edge-parallel within core):
- Node rows sharded across 8 cores by dst range (6250/core).
- GEMM1 folds el/er attention dots into extended W columns; row = 640
  [msg 512 | el 8 | er 8 | pad]; AllGather -> replicated hfull.
- Edge phase per 128-dst-block: src rows fetched with dma_gather over 4
  SWDGE queues; gathers software-pipelined one group ahead of compute.
- msg columns stored (f,h)-major (host-permuted W1/b1/W2-rows) so the
  attention-weight multiply and normalize run in DVE 2x packed mode.
- ohT built by TensorE transpose of oh (no dstlocT input, no 2nd
  is_equal); PSUM->SBUF copies on the scalar engine to unload DVE.
- Segment softmax numerator/denominator accumulate per block with
  one-hot matmuls into PSUM (exp(e) written into the slab el slot so
  one rhs covers msg[256:512]+a).
- GEMM2 fused per block (TensorE transpose + matmul) -> bounce_h2.
- No segment-max subtraction (reference's emax cancels exactly).
"""

import sys
import time
import hashlib

for _p in ("/opt/trn_rl_repo", "/root/.axon_site", "/root/.axon_site/_ro/trn_rl_repo"):
    if _p not in sys.path:
        sys.path.insert(0, _p)

import numpy as np
import ml_dtypes

import concourse.bass as bass
import concourse.mybir as mybir
import concourse.tile as tile
from concourse import bacc
from concourse import bass_utils

F32 = mybir.dt.float32
BF16 = mybir.dt.bfloat16
I16 = mybir.dt.int16
BF = ml_dtypes.bfloat16

NEG_SLOPE = 0.2
DENOM_EPS = 1e-30
NQ = 4  # SWDGE queues for gather round-robin


# ---------------------------------------------------------------------------
# configuration


class Cfg:
    def __init__(self, N, E, F, HID, HEADS, NC=8, GBLK=1):
        self.N, self.E, self.F, self.HID, self.HEADS, self.NC = N, E, F, HID, HEADS, NC
        self.HALF = N // 2
        self.NB = N // NC                      # nodes per core
        self.NBLK = (self.NB + 127) // 128     # dst blocks per core
        self.NBpad = self.NBLK * 128
        self.GBLK = GBLK
        self.KT1 = (F + 127) // 128
        self.Fpad = self.KT1 * 128
        self.C1 = HEADS * HID                  # 512
        self.KT2 = (self.C1 + 127) // 128      # 4
        self.C1pad = self.KT2 * 128
        self.C2 = HID                          # 64
        # layer-1 row: [msg 512 | el 8 | er 8 | pad 112] -> 640 (1280 B)
        self.RW1 = 640
        self.EL1 = self.C1                     # el offset
        self.ER1 = self.C1 + HEADS             # er offset
        # layer-2 row: [msg 64 | el 1 | er 1 | pad] -> 128 (256 B)
        self.RW2 = 128
        self.EL2 = self.C2
        self.ER2 = self.C2 + 1
        assert self.HALF < 32768 and self.NB < 32768


# ---------------------------------------------------------------------------
# host-side edge preprocessing


def wrap_idxs(idx: np.ndarray) -> np.ndarray:
    """int16 idx list (len = nt*128) -> [128, nt*8] wrapped-16, replicated."""
    n = len(idx)
    assert n % 128 == 0
    m = n // 16
    out = np.zeros((16, m), dtype=np.int16)
    out[np.arange(n) % 16, np.arange(n) // 16] = idx
    return np.tile(out, (8, 1))


def make_plan(src, dst, cfg: Cfg):
    """Static tile plan (shared across cores) + per-core edge tensors."""
    NC, NB, NBLK, HALF, GBLK = cfg.NC, cfg.NB, cfg.NBLK, cfg.HALF, cfg.GBLK

    per_core = []
    counts = np.zeros((NC, NBLK, 2), dtype=np.int64)
    for r in range(NC):
        sel = np.nonzero((dst >= r * NB) & (dst < (r + 1) * NB))[0]
        es = src[sel].astype(np.int64)
        ed = (dst[sel] - r * NB).astype(np.int64)
        blk = ed >> 7
        half = (es >= HALF).astype(np.int64)
        key = blk * 2 + half
        o = np.argsort(key, kind="stable")
        es, ed, key = es[o], ed[o], key[o]
        cnt = np.bincount(key, minlength=NBLK * 2)
        counts[r] = cnt.reshape(NBLK, 2)
        per_core.append((es, ed, np.concatenate([[0], np.cumsum(cnt)])))

    cmax = counts.max(axis=0)                  # [NBLK, 2]
    ntile = (cmax + 127) // 128

    groups = []
    olo = ohi = oall = 0
    for g0 in range(0, NBLK, GBLK):
        blocks = list(range(g0, min(g0 + GBLK, NBLK)))
        ntlo = int(sum(ntile[b, 0] for b in blocks))
        nthi = int(sum(ntile[b, 1] for b in blocks))
        nt = ntlo + nthi
        tile_blk = []
        for b in blocks:
            tile_blk += [b] * int(ntile[b, 0])
        for b in blocks:
            tile_blk += [b] * int(ntile[b, 1])
        groups.append(dict(blocks=blocks, ntlo=ntlo, nthi=nthi, nt=nt,
                           tile_blk=tile_blk, olo=olo, ohi=ohi, oall=oall))
        olo, ohi, oall = olo + ntlo, ohi + nthi, oall + nt

    # per-core flat arrays in group/tile order
    core_data = []
    for r in range(NC):
        es, ed, cum = per_core[r]
        lo_idx, hi_idx, dvals = [], [], []
        for g in groups:
            for h, acc in ((0, lo_idx), (1, hi_idx)):
                for b in g["blocks"]:
                    k = b * 2 + h
                    s, e = cum[k], cum[k + 1]
                    n_pad = int(ntile[b, h]) * 128
                    v = es[s:e] - (HALF if h else 0)
                    v = np.concatenate([v, np.zeros(n_pad - len(v), np.int64)])
                    acc.append(v)
                    d = np.concatenate([ed[s:e],
                                        np.full(n_pad - (e - s), -1, np.int64)])
                    dv = np.where(d >= 0, d & 127, 999).astype(np.float32)
                    dvals.append(dv)
        lo = np.concatenate(lo_idx) if lo_idx else np.zeros(0, np.int64)
        hi = np.concatenate(hi_idx) if hi_idx else np.zeros(0, np.int64)
        dv = np.concatenate(dvals)
        # wrap per group; dl in [128, nt] layout
        glo, ghi, gdl = [], [], []
        plo = phi = pall = 0
        for g in groups:
            nlo, nhi, nt = g["ntlo"] * 128, g["nthi"] * 128, g["nt"] * 128
            glo.append(wrap_idxs(lo[plo:plo + nlo].astype(np.int16)))
            ghi.append(wrap_idxs(hi[phi:phi + nhi].astype(np.int16)))
            dvg = dv[pall:pall + nt]
            gdl.append(dvg.reshape(g["nt"], 128).T)
            plo, phi, pall = plo + nlo, phi + nhi, pall + nt
        core_data.append(dict(
            g1lo=np.concatenate(glo, axis=1) if glo else np.zeros((128, 0), np.int16),
            g1hi=np.concatenate(ghi, axis=1) if ghi else np.zeros((128, 0), np.int16),
            dstloc=np.concatenate(gdl, axis=1).astype(BF),
        ))

    plan = dict(groups=groups, ntile=ntile,
                TLO=int(sum(g["ntlo"] for g in groups)),
                THI=int(sum(g["nthi"] for g in groups)),
                TT=int(sum(g["nt"] for g in groups)))
    return plan, core_data


MAX_GIDX = 1024  # HW limit: dma_gather crashes above 1024 indices

_gq = [0]


def chunked_gather(nc, out_slab, in_ap, idx_tile, t0, ntc, elem, step=None,
                   nq=1):
    CT = MAX_GIDX // 128
    for q0 in range(0, ntc, CT):
        qn = min(CT, ntc - q0)
        kw = dict(elem_step=step) if step else {}
        nc.gpsimd.dma_gather(
            out_ap=out_slab[:, t0 + q0:t0 + q0 + qn, :], in_ap=in_ap,
            idxs_ap=idx_tile[:, q0 * 8:(q0 + qn) * 8],
            num_idxs=qn * 128, num_idxs_reg=qn * 128, elem_size=elem,
            queue_num=_gq[0] % nq, **kw)
        _gq[0] += 1


# ---------------------------------------------------------------------------
# program builder


def build_program(cfg: Cfg, plan, reps=1, phases="ABCEF", force_shared=False,
                  nq=NQ):
    ag_space = "Shared" if (reps == 1 or force_shared) else "Local"
    c = cfg
    H = c.HEADS
    nc = bacc.Bacc("TRN2", target_bir_lowering=False, debug=False,
                   num_devices=c.NC, num_swdge_queues=nq)
    _gq[0] = 0

    dt = nc.dram_tensor
    featsT = dt("featsT", [c.KT1, 128, c.NBpad], BF16, kind="ExternalInput")
    w1ext = dt("w1ext", [c.KT1, 128, c.RW1], BF16, kind="ExternalInput")
    w2ext = dt("w2ext", [c.KT2, 128, c.RW2], BF16, kind="ExternalInput")
    b1rep = dt("b1rep", [128, c.C1], BF16, kind="ExternalInput")
    b2rep = dt("b2rep", [128, c.C2], F32, kind="ExternalInput")
    iota = dt("iota", [128, 128], BF16, kind="ExternalInput")
    iotap = dt("iotap", [128, 1], BF16, kind="ExternalInput")
    g1lo = dt("g1lo", [128, max(1, plan["TLO"] * 8)], I16, kind="ExternalInput")
    g1hi = dt("g1hi", [128, max(1, plan["THI"] * 8)], I16, kind="ExternalInput")
    dstloc = dt("dstloc", [128, plan["TT"]], BF16, kind="ExternalInput")
    out = dt("out", [c.NB, c.C2], F32, kind="ExternalOutput")

    groups = plan["groups"]

    with tile.TileContext(nc) as tc:
        with tc.tile_pool(name="dram", bufs=1, space="DRAM") as dram:
            bounce_h = dram.tile([c.NB, c.RW1], BF16)
            bounce_h2 = dram.tile([c.NB, c.RW2], BF16)
            # one-hot stash: built in C, reused in F (same dl both layers)
            ohst = dram.tile([128, max(1, plan["TT"]) * 128], BF16)
            ohTst = dram.tile([128, max(1, plan["TT"]) * 128], BF16)

            for _rep in range(reps):
                # Shared tiles allow a single writer only -> per-rep tiles.
                hfull = dram.tile([c.N, c.RW1], BF16, addr_space=ag_space)
                h2full = dram.tile([c.N, c.RW2], BF16, addr_space=ag_space)

                if "A" in phases:
                    # ------------- phase A: GEMM1 -> bounce_h -------------
                    with (
                        nc.named_scope("phaseA_gemm1"),
                        tc.tile_pool(name="ga", bufs=1) as cpool,
                        tc.tile_pool(name="gaw", bufs=2) as wpool,
                        tc.tile_pool(name="gap", bufs=2, space="PSUM") as pspool,
                    ):
                        w1sb = cpool.tile([128, c.KT1, c.RW1], BF16)
                        ftsb = cpool.tile([128, c.KT1, c.NBpad], BF16)
                        nc.sync.dma_start(w1sb[:], w1ext[:].rearrange("k p w -> p k w"))
                        nc.sync.dma_start(ftsb[:], featsT[:].rearrange("k p w -> p k w"))
                        for ntb in range(c.NBLK):
                            pa = pspool.tile([128, 512], F32, tag="pa", space="PSUM")
                            pb = pspool.tile([128, c.RW1 - 512], F32, tag="pb",
                                             space="PSUM")
                            for k in range(c.KT1):
                                lhsT = ftsb[:, k, ntb * 128:(ntb + 1) * 128]
                                nc.tensor.matmul(pa[:], lhsT, w1sb[:, k, 0:512],
                                                 start=(k == 0), stop=(k == c.KT1 - 1))
                                nc.tensor.matmul(pb[:], lhsT,
                                                 w1sb[:, k, 512:c.RW1],
                                                 start=(k == 0), stop=(k == c.KT1 - 1))
                            ht = wpool.tile([128, c.RW1], BF16, tag="ht")
                            nc.scalar.copy(ht[:, 0:512], pa[:])
                            nc.scalar.copy(ht[:, 512:c.RW1], pb[:])
                            rows = min(128, c.NB - ntb * 128)
                            nc.sync.dma_start(bounce_h[ntb * 128:ntb * 128 + rows, :],
                                              ht[:rows, :])

                if "B" in phases:
                    # ------------- phase B: AllGather h -------------
                    with nc.named_scope("phaseB_ag1"):
                        nc.gpsimd.collective_compute(
                            "AllGather", mybir.AluOpType.bypass,
                            replica_groups=[list(range(c.NC))],
                            ins=[bounce_h.opt()], outs=[hfull.opt()],
                        )

                if "g" in phases:
                    # ---- phase g: layer-1 gathers ONLY (timing isolation) ----
                    with (
                        nc.named_scope("phaseG_gatheronly"),
                        tc.tile_pool(name="g1", bufs=3) as sb,
                    ):
                        for g in groups:
                            ntlo, nthi, nt = g["ntlo"], g["nthi"], g["nt"]
                            if nt == 0:
                                continue
                            slab = sb.tile([128, nt, c.RW1], BF16, tag="slab")
                            if ntlo:
                                ilo = sb.tile([128, ntlo * 8], I16, tag="ilo")
                                nc.sync.dma_start(
                                    ilo[:], g1lo[:, g["olo"] * 8:(g["olo"] + ntlo) * 8])
                                chunked_gather(nc, slab, hfull[0:c.HALF, :], ilo,
                                               0, ntlo, c.RW1, nq=nq)
                            if nthi:
                                ihi = sb.tile([128, nthi * 8], I16, tag="ihi")
                                nc.sync.dma_start(
                                    ihi[:], g1hi[:, g["ohi"] * 8:(g["ohi"] + nthi) * 8])
                                chunked_gather(nc, slab, hfull[c.HALF:c.N, :], ihi,
                                               ntlo, nthi, c.RW1, nq=nq)

                if "C" in phases or "c" in phases:
                    # ------- phase C: layer-1 edge phase + fused GEMM2 -------
                    # "c" variant: sequential slab fill to time compute side
                    do_gather = "C" in phases
                    with (
                        nc.named_scope("phaseC_edge1"),
                        tc.tile_pool(name="ec", bufs=1) as cst,
                        tc.tile_pool(name="e1g", bufs=3) as sbg,
                        tc.tile_pool(name="e1", bufs=2) as sb,
                        tc.tile_pool(name="e1n", bufs=2) as nsb,
                        tc.tile_pool(name="e1p", bufs=2, space="PSUM") as ps,
                        tc.tile_pool(name="e1e", bufs=1, space="PSUM") as pse,
                        tc.tile_pool(name="e1q", bufs=1, space="PSUM") as psq,
                        tc.tile_pool(name="e1t", bufs=2, space="PSUM") as pst,
                    ):
                        iosb = cst.tile([128, 128], BF16)
                        nc.sync.dma_start(iosb[:], iota[:])
                        iopsb = cst.tile([128, 1], BF16)
                        nc.sync.dma_start(iopsb[:], iotap[:])
                        b1sb = cst.tile([128, c.C1], BF16)
                        nc.sync.dma_start(b1sb[:], b1rep[:])
                        ident = cst.tile([128, 128], BF16)
                        nc.vector.memset(ident[:], 0.0)
                        nc.vector.tensor_tensor(
                            out=ident[:], in0=iosb[:],
                            in1=iopsb[:].broadcast_to([128, 128]),
                            op=mybir.AluOpType.is_equal)
                        w2sb = cst.tile([128, c.KT2, c.RW2], BF16)
                        nc.sync.dma_start(w2sb[:], w2ext[:].rearrange("k p w -> p k w"))
                        # relu(b1) -> xb0; its fused-GEMM2 row (for empty blocks)
                        xb0 = cst.tile([128, c.C1], BF16)
                        nc.scalar.activation(xb0[:], b1sb[:],
                                             mybir.ActivationFunctionType.Relu)
                        xt20 = cst.tile([128, c.KT2, 128], BF16)
                        h2t0 = cst.tile([128, c.RW2], BF16)
                        pt0 = pst.tile([128, 128], BF16, tag="pt", space="PSUM")
                        for k in range(c.KT2):
                            nc.tensor.transpose(
                                pt0[:], xb0[:, k * 128:(k + 1) * 128], ident[:])
                            nc.scalar.copy(xt20[:, k, :], pt0[:])
                        pc20 = psq.tile([128, c.RW2], F32, tag="pc2", space="PSUM")
                        for k in range(c.KT2):
                            nc.tensor.matmul(pc20[:], xt20[:, k, :], w2sb[:, k, :],
                                             start=(k == 0), stop=(k == c.KT2 - 1))
                        nc.scalar.copy(h2t0[:], pc20[:])

                        def c_loads(g):
                            ntlo, nthi, nt = g["ntlo"], g["nthi"], g["nt"]
                            slab = sbg.tile([128, nt, c.RW1], BF16, tag="slab")
                            dl = sbg.tile([128, nt], BF16, tag="dl")
                            erb = sbg.tile([128, c.GBLK, H], BF16, tag="erb")
                            nc.sync.dma_start(
                                dl[:], dstloc[:, g["oall"]:g["oall"] + nt])
                            nc.vector.memset(erb[:], 0.0)
                            for j, b in enumerate(g["blocks"]):
                                rows = min(128, c.NB - b * 128)
                                nc.sync.dma_start(
                                    erb[:rows, j, :],
                                    bounce_h[b * 128:b * 128 + rows,
                                             c.ER1:c.ER1 + H])
                            if do_gather:
                                if ntlo:
                                    ilo = sbg.tile([128, ntlo * 8], I16, tag="ilo")
                                    nc.sync.dma_start(
                                        ilo[:],
                                        g1lo[:, g["olo"] * 8:(g["olo"] + ntlo) * 8])
                                    chunked_gather(nc, slab, hfull[0:c.HALF, :],
                                                   ilo, 0, ntlo, c.RW1, nq=nq)
                                if nthi:
                                    ihi = sbg.tile([128, nthi * 8], I16, tag="ihi")
                                    nc.sync.dma_start(
                                        ihi[:],
                                        g1hi[:, g["ohi"] * 8:(g["ohi"] + nthi) * 8])
                                    chunked_gather(nc, slab, hfull[c.HALF:c.N, :],
                                                   ihi, ntlo, nthi, c.RW1, nq=nq)
                            else:
                                nc.sync.dma_start(
                                    slab[:],
                                    hfull[0:nt * 128, :].rearrange(
                                        "(t p) e -> p t e", p=128))
                            return slab, dl, erb

                        def c_compute(g, slab, dl, erb):
                            nt = g["nt"]
                            # one-hot [e, d]; ohT via TensorE transpose
                            oh = sb.tile([128, nt, 128], BF16, tag="oh")
                            ohT = sb.tile([128, nt, 128], BF16, tag="ohT")
                            nc.vector.tensor_tensor(
                                out=oh[:],
                                in0=iosb[:, None, :].broadcast_to([128, nt, 128]),
                                in1=dl[:, :, None].broadcast_to([128, nt, 128]),
                                op=mybir.AluOpType.is_equal)
                            erg = pse.tile([128, nt, H], F32, tag="erg",
                                           space="PSUM")
                            for t, tb in enumerate(g["tile_blk"]):
                                j = g["blocks"].index(tb)
                                ptT = pst.tile([128, 128], BF16, tag="pt",
                                               space="PSUM")
                                nc.tensor.transpose(ptT[:], oh[:, t, :], ident[:])
                                nc.scalar.copy(ohT[:, t, :], ptT[:])
                                nc.tensor.matmul(erg[:, t, :], ohT[:, t, :],
                                                 erb[:, j, :],
                                                 start=True, stop=True)
                            # stash one-hots for phase F reuse
                            o0 = g["oall"] * 128
                            nc.sync.dma_start(
                                ohst[:, o0:o0 + nt * 128],
                                oh[:].rearrange("p t e -> p (t e)"))
                            nc.sync.dma_start(
                                ohTst[:, o0:o0 + nt * 128],
                                ohT[:].rearrange("p t e -> p (t e)"))
                            # e = el + er ; leaky-relu ; exp -> a (el slot)
                            et = sb.tile([128, nt, H], F32, tag="et")
                            e2 = sb.tile([128, nt, H], F32, tag="e2")
                            nc.vector.tensor_tensor(
                                out=et[:], in0=slab[:, :, c.EL1:c.EL1 + H],
                                in1=erg[:], op=mybir.AluOpType.add)
                            nc.vector.scalar_tensor_tensor(
                                out=e2[:], in0=et[:], scalar=NEG_SLOPE,
                                in1=et[:], op0=mybir.AluOpType.mult,
                                op1=mybir.AluOpType.max)
                            nc.scalar.activation(slab[:, :, c.EL1:c.EL1 + H],
                                                 e2[:],
                                                 mybir.ActivationFunctionType.Exp)
                            # msg = h * a ((f,h)-major: contiguous 8-runs)
                            nc.vector.tensor_tensor(
                                out=slab[:, :, 0:c.C1].rearrange(
                                    "p t (f h) -> p t f h", h=H),
                                in0=slab[:, :, 0:c.C1].rearrange(
                                    "p t (f h) -> p t f h", h=H),
                                in1=slab[:, :, c.EL1:c.EL1 + H][:, :, None, :]
                                .broadcast_to([128, nt, c.HID, H]),
                                op=mybir.AluOpType.mult)
                            # per-block accumulate + normalize + fused GEMM2
                            for b in g["blocks"]:
                                tlist = [t for t, tb in enumerate(g["tile_blk"])
                                         if tb == b]
                                rows = min(128, c.NB - b * 128)
                                if not tlist:
                                    nc.sync.dma_start(
                                        bounce_h2[b * 128:b * 128 + rows, :],
                                        h2t0[:rows, :])
                                    continue
                                pa = ps.tile([128, 256], F32, tag="pa",
                                             space="PSUM")
                                pb = ps.tile([128, 264], F32, tag="pb",
                                             space="PSUM")
                                for j, t in enumerate(tlist):
                                    st, sp = (j == 0), (j == len(tlist) - 1)
                                    nc.tensor.matmul(pa[:], oh[:, t, :],
                                                     slab[:, t, 0:256],
                                                     start=st, stop=sp)
                                    nc.tensor.matmul(pb[:], oh[:, t, :],
                                                     slab[:, t, 256:520],
                                                     start=st, stop=sp)
                                dg = nsb.tile([128, H], F32, tag="dg")
                                rd = nsb.tile([128, H], F32, tag="rd")
                                nc.vector.tensor_scalar_max(dg[:], pb[:, 256:264],
                                                            DENOM_EPS)
                                nc.vector.reciprocal(rd[:], dg[:])
                                # normalize ((f,h)-major) + bias -> bf16
                                xt = nsb.tile([128, c.C1], BF16, tag="xt")
                                F2 = 256 // H  # f-cols per half
                                nc.vector.tensor_tensor(
                                    out=xt[:, 0:256].rearrange(
                                        "p (f h) -> p f h", h=H),
                                    in0=pa[:].rearrange("p (f h) -> p f h", h=H),
                                    in1=rd[:, None, :].broadcast_to([128, F2, H]),
                                    op=mybir.AluOpType.mult)
                                nc.vector.tensor_tensor(
                                    out=xt[:, 256:512].rearrange(
                                        "p (f h) -> p f h", h=H),
                                    in0=pb[:, 0:256].rearrange(
                                        "p (f h) -> p f h", h=H),
                                    in1=rd[:, None, :].broadcast_to([128, F2, H]),
                                    op=mybir.AluOpType.mult)
                                nc.vector.tensor_tensor(out=xt[:], in0=xt[:],
                                                        in1=b1sb[:],
                                                        op=mybir.AluOpType.add)
                                xb = nsb.tile([128, c.C1], BF16, tag="xb")
                                nc.scalar.activation(xb[:], xt[:],
                                                     mybir.ActivationFunctionType.Relu)
                                # fused GEMM2 for this block
                                xt2 = nsb.tile([128, c.KT2, 128], BF16, tag="xt2")
                                for k in range(c.KT2):
                                    pt = pst.tile([128, 128], BF16, tag="pt",
                                                  space="PSUM")
                                    nc.tensor.transpose(
                                        pt[:], xb[:, k * 128:(k + 1) * 128],
                                        ident[:])
                                    nc.scalar.copy(xt2[:, k, :], pt[:])
                                pc2 = psq.tile([128, c.RW2], F32, tag="pc2",
                                               space="PSUM")
                                for k in range(c.KT2):
                                    nc.tensor.matmul(pc2[:], xt2[:, k, :],
                                                     w2sb[:, k, :],
                                                     start=(k == 0),
                                                     stop=(k == c.KT2 - 1))
                                h2t = nsb.tile([128, c.RW2], BF16, tag="h2t")
                                nc.scalar.copy(h2t[:], pc2[:])
                                nc.sync.dma_start(
                                    bounce_h2[b * 128:b * 128 + rows, :],
                                    h2t[:rows, :])

                        pend = None
                        for g in groups:
                            if g["nt"] == 0:
                                for b in g["blocks"]:
                                    rows = min(128, c.NB - b * 128)
                                    nc.sync.dma_start(
                                        bounce_h2[b * 128:b * 128 + rows, :],
                                        h2t0[:rows, :])
                                continue
                            tls = c_loads(g)
                            if pend is not None:
                                c_compute(pend[0], *pend[1])
                            pend = (g, tls)
                        if pend is not None:
                            c_compute(pend[0], *pend[1])

                if "E" in phases:
                    # ------------- phase E: AllGather h2 -------------
                    with nc.named_scope("phaseE_ag2"):
                        nc.gpsimd.collective_compute(
                            "AllGather", mybir.AluOpType.bypass,
                            replica_groups=[list(range(c.NC))],
                            ins=[bounce_h2.opt()], outs=[h2full.opt()],
                        )

                if "F" in phases:
                    # ------------- phase F: layer-2 edge phase -------------
                    with (
                        nc.named_scope("phaseF_edge2"),
                        tc.tile_pool(name="fc", bufs=1) as cst,
                        tc.tile_pool(name="f1g", bufs=3) as sbg,
                        tc.tile_pool(name="f1", bufs=2) as sb,
                        tc.tile_pool(name="f1n", bufs=2) as nsb,
                        tc.tile_pool(name="f1p", bufs=2, space="PSUM") as ps,
                    ):
                        b2sb = cst.tile([128, c.C2], F32)
                        nc.sync.dma_start(b2sb[:], b2rep[:])
                        ot0 = cst.tile([128, c.C2], F32)
                        nc.vector.tensor_copy(ot0[:], b2sb[:])

                        def f_loads(g):
                            ntlo, nthi, nt = g["ntlo"], g["nthi"], g["nt"]
                            slab = sbg.tile([128, nt, c.RW2], BF16, tag="slab2")
                            oh = sbg.tile([128, nt, 128], BF16, tag="oh2")
                            ohT = sbg.tile([128, nt, 128], BF16, tag="ohT2")
                            o0 = g["oall"] * 128
                            nc.sync.dma_start(
                                oh[:].rearrange("p t e -> p (t e)"),
                                ohst[:, o0:o0 + nt * 128])
                            nc.sync.dma_start(
                                ohT[:].rearrange("p t e -> p (t e)"),
                                ohTst[:, o0:o0 + nt * 128])
                            erb = sbg.tile([128, c.GBLK, 1], BF16, tag="erb2")
                            nc.vector.memset(erb[:], 0.0)
                            for j, b in enumerate(g["blocks"]):
                                rows = min(128, c.NB - b * 128)
                                nc.sync.dma_start(
                                    erb[:rows, j, :],
                                    bounce_h2[b * 128:b * 128 + rows,
                                              c.ER2:c.ER2 + 1])
                            if ntlo:
                                ilo = sbg.tile([128, ntlo * 8], I16, tag="ilo2")
                                nc.sync.dma_start(
                                    ilo[:], g1lo[:, g["olo"] * 8:(g["olo"] + ntlo) * 8])
                                chunked_gather(nc, slab, h2full[0:c.HALF, :],
                                               ilo, 0, ntlo, c.RW2, nq=nq)
                            if nthi:
                                ihi = sbg.tile([128, nthi * 8], I16, tag="ihi2")
                                nc.sync.dma_start(
                                    ihi[:], g1hi[:, g["ohi"] * 8:(g["ohi"] + nthi) * 8])
                                chunked_gather(nc, slab, h2full[c.HALF:c.N, :],
                                               ihi, ntlo, nthi, c.RW2, nq=nq)
                            return slab, oh, ohT, erb

                        def f_compute(g, slab, oh, ohT, erb):
                            nt = g["nt"]
                            erg = ps.tile([128, nt, 1], F32, tag="erg2",
                                          space="PSUM")
                            for t, tb in enumerate(g["tile_blk"]):
                                j = g["blocks"].index(tb)
                                nc.tensor.matmul(erg[:, t, :], ohT[:, t, :],
                                                 erb[:, j, :],
                                                 start=True, stop=True)
                            et = sb.tile([128, nt, 1], F32, tag="et2")
                            e2 = sb.tile([128, nt, 1], F32, tag="e22")
                            nc.vector.tensor_tensor(
                                out=et[:], in0=slab[:, :, c.EL2:c.EL2 + 1],
                                in1=erg[:], op=mybir.AluOpType.add)
                            nc.vector.scalar_tensor_tensor(
                                out=e2[:], in0=et[:], scalar=NEG_SLOPE,
                                in1=et[:], op0=mybir.AluOpType.mult,
                                op1=mybir.AluOpType.max)
                            nc.scalar.activation(slab[:, :, c.EL2:c.EL2 + 1],
                                                 e2[:],
                                                 mybir.ActivationFunctionType.Exp)
                            nc.vector.tensor_tensor(
                                out=slab[:, :, 0:c.C2],
                                in0=slab[:, :, 0:c.C2],
                                in1=slab[:, :, c.EL2:c.EL2 + 1].broadcast_to(
                                    [128, nt, c.C2]),
                                op=mybir.AluOpType.mult)
                            for b in g["blocks"]:
                                tlist = [t for t, tb in enumerate(g["tile_blk"])
                                         if tb == b]
                                rows = min(128, c.NB - b * 128)
                                if not tlist:
                                    nc.sync.dma_start(out[b * 128:b * 128 + rows, :],
                                                      ot0[:rows, :])
                                    continue
                                pc = ps.tile([128, c.C2 + 1], F32, tag="pc",
                                             space="PSUM")
                                for j, t in enumerate(tlist):
                                    nc.tensor.matmul(pc[:], oh[:, t, :],
                                                     slab[:, t, 0:c.C2 + 1],
                                                     start=(j == 0),
                                                     stop=(j == len(tlist) - 1))
                                dg = nsb.tile([128, 1], F32, tag="dg2")
                                rd = nsb.tile([128, 1], F32, tag="rd2")
                                nc.vector.tensor_scalar_max(dg[:],
                                                            pc[:, c.C2:c.C2 + 1],
                                                            DENOM_EPS)
                                nc.vector.reciprocal(rd[:], dg[:])
                                ot = nsb.tile([128, c.C2], F32, tag="ot")
                                nc.vector.tensor_scalar(
                                    out=ot[:], in0=pc[:, 0:c.C2],
                                    scalar1=rd[:, 0:1],
                                    scalar2=None, op0=mybir.AluOpType.mult)
                                nc.vector.tensor_tensor(out=ot[:], in0=ot[:],
                                                        in1=b2sb[:],
                                                        op=mybir.AluOpType.add)
                                nc.sync.dma_start(out[b * 128:b * 128 + rows, :],
                                                  ot[:rows, :])

                        pend = None
                        for g in groups:
                            if g["nt"] == 0:
                                for b in g["blocks"]:
                                    rows = min(128, c.NB - b * 128)
                                    nc.sync.dma_start(
                                        out[b * 128:b * 128 + rows, :],
                                        ot0[:rows, :])
                                continue
                            tls = f_loads(g)
                            if pend is not None:
                                f_compute(pend[0], *pend[1])
                            pend = (g, tls)
                        if pend is not None:
                            f_compute(pend[0], *pend[1])

    nc.compile()
    return nc


# ---------------------------------------------------------------------------
# host orchestration


def make_inputs(inputs, cfg: Cfg, plan, core_data):
    c = cfg
    feats = np.asarray(inputs["feats"], np.float32)
    W1 = np.asarray(inputs["W1"], np.float32)
    al1 = np.asarray(inputs["attn_l1"], np.float32)
    ar1 = np.asarray(inputs["attn_r1"], np.float32)
    b1 = np.asarray(inputs["b1"], np.float32)
    W2 = np.asarray(inputs["W2"], np.float32)
    al2 = np.asarray(inputs["attn_l2"], np.float32)
    ar2 = np.asarray(inputs["attn_r2"], np.float32)
    b2 = np.asarray(inputs["b2"], np.float32)

    H, HID = c.HEADS, c.HID
    # (f,h)-major permutation: new col j = f*H + h <- old col h*HID + f
    jj = np.arange(c.C1)
    perm = (jj % H) * HID + (jj // H)

    W1r = W1.reshape(c.F, H, HID)
    Wl1 = np.einsum("khd,hd->kh", W1r, al1)
    Wr1 = np.einsum("khd,hd->kh", W1r, ar1)
    w1e = np.zeros((c.Fpad, c.RW1), np.float32)
    w1e[:c.F, 0:c.C1] = W1[:, perm]
    w1e[:c.F, c.EL1:c.EL1 + H] = Wl1
    w1e[:c.F, c.ER1:c.ER1 + H] = Wr1
    w1e = w1e.reshape(c.KT1, 128, c.RW1).astype(BF)

    Wl2 = W2 @ al2[0]
    Wr2 = W2 @ ar2[0]
    w2e = np.zeros((c.C1pad, c.RW2), np.float32)
    w2e[:c.C1, 0:c.C2] = W2[perm, :]
    w2e[:c.C1, c.EL2] = Wl2[perm]
    w2e[:c.C1, c.ER2] = Wr2[perm]
    w2e = w2e.reshape(c.KT2, 128, c.RW2).astype(BF)

    b1r = np.tile(b1[perm][None, :], (128, 1)).astype(BF)
    b2r = np.tile(b2[None, :], (128, 1)).astype(np.float32)
    iot = np.tile(np.arange(128, dtype=np.float32)[None, :], (128, 1)).astype(BF)
    iop = np.arange(128, dtype=np.float32)[:, None].astype(BF)

    in_maps = []
    for r in range(c.NC):
        ft = np.zeros((c.Fpad, c.NBpad), np.float32)
        ft[:c.F, :c.NB] = feats[r * c.NB:(r + 1) * c.NB].T
        cd = core_data[r]
        in_maps.append(dict(
            featsT=ft.reshape(c.KT1, 128, c.NBpad).astype(BF),
            w1ext=w1e, w2ext=w2e, b1rep=b1r, b2rep=b2r, iota=iot, iotap=iop,
            g1lo=cd["g1lo"] if cd["g1lo"].shape[1] else
                np.zeros((128, 1), np.int16),
            g1hi=cd["g1hi"] if cd["g1hi"].shape[1] else
                np.zeros((128, 1), np.int16),
            dstloc=cd["dstloc"],
        ))
    return in_maps


_CACHE = {}


def _get_compiled(inputs, cfg):
    src = np.asarray(inputs["src"], np.int64)
    dst = np.asarray(inputs["dst"], np.int64)
    key = hashlib.sha1(np.ascontiguousarray(src).tobytes()
                       + np.ascontiguousarray(dst).tobytes()).hexdigest()
    if key not in _CACHE:
        plan, core_data = make_plan(src, dst, cfg)
        nc = build_program(cfg, plan)
        _CACHE[key] = (nc, plan, core_data)
    return _CACHE[key]


def kernel(**inputs) -> np.ndarray:
    feats = np.asarray(inputs["feats"])
    H, HID = np.asarray(inputs["attn_l1"]).shape
    cfg = Cfg(N=feats.shape[0], E=np.asarray(inputs["src"]).shape[0],
              F=feats.shape[1], HID=HID, HEADS=H)
    nc, plan, core_data = _get_compiled(inputs, cfg)
    in_maps = make_inputs(inputs, cfg, plan, core_data)
    res = bass_utils.run_bass_kernel_spmd(
        nc, in_maps, core_ids=list(range(cfg.NC)), trace=False)
    return np.concatenate([res.results[r]["out"] for r in range(cfg.NC)], axis=0)


# ---------------------------------------------------------------------------
# device-resident timing runner


class Runner:
    """Compiled SPMD executable with device-resident inputs."""

    def __init__(self, nc, in_maps, n_cores):
        import jax
        from jax.experimental.shard_map import shard_map
        from jax.sharding import Mesh, PartitionSpec
        from concourse import bass2jax, mybir as mb

        bass2jax.install_neuronx_cc_hook()
        pid_name = (nc.partition_id_tensor.name
                    if nc.partition_id_tensor else None)
        in_names, out_names, out_avals, zero_outs = [], [], [], []
        for alloc in nc.m.functions[0].allocations:
            if not isinstance(alloc, mb.MemoryLocationSet):
                continue
            name = alloc.memorylocations[0].name
            if alloc.kind == "ExternalInput":
                if name != pid_name:
                    in_names.append(name)
            elif alloc.kind == "ExternalOutput":
                out_names.append(name)
                out_avals.append(jax.core.ShapedArray(
                    tuple(alloc.tensor_shape), mb.dt.np(alloc.dtype)))
                zero_outs.append(np.zeros(alloc.tensor_shape,
                                          mb.dt.np(alloc.dtype)))
        n_params = len(in_names)
        all_names = in_names + out_names

        if pid_name is not None:
            all_names = all_names + [pid_name]

        def _body(*args):
            operands = list(args)
            if pid_name is not None:
                operands.append(bass2jax.partition_id_tensor())
            outs = bass2jax._bass_exec_p.bind(
                *operands, out_avals=tuple(out_avals), in_names=tuple(all_names),
                out_names=tuple(out_names), lowering_input_output_aliases=(),
                sim_require_finite=True, sim_require_nnan=True, nc=nc)
            return tuple(outs)

        devices = jax.devices()[:n_cores]
        mesh = Mesh(np.asarray(devices), ("core",))
        specs = (PartitionSpec("core"),) * (n_params + len(out_names))
        self._fn = jax.jit(shard_map(_body, mesh=mesh, in_specs=specs,
                                     out_specs=(PartitionSpec("core"),) * len(out_names),
                                     check_rep=False), keep_unused=True)
        concat_in = [np.concatenate([np.asarray(in_maps[c][nm])
                                     for c in range(n_cores)], axis=0)
                     for nm in in_names]
        concat_zero = [np.zeros((n_cores * z.shape[0], *z.shape[1:]), z.dtype)
                       for z in zero_outs]
        self._args = [jax.device_put(a) for a in concat_in + concat_zero]
        self.out_names, self.out_avals, self.n_cores = out_names, out_avals, n_cores

    def run(self):
        outs = self._fn(*self._args)
        for o in outs:
            o.block_until_ready()
        return outs

    def results(self):
        import numpy as _np
        outs = self.run()
        return [
            {nm: _np.asarray(outs[i]).reshape(self.n_cores,
                                              *self.out_avals[i].shape)[c]
             for i, nm in enumerate(self.out_names)}
            for c in range(self.n_cores)
        ]

    def time_ns(self, iters=12, warmup=3):
        for _ in range(warmup):
            self.run()
        best = float("inf")
        for _ in range(iters):
            t0 = time.perf_counter()
            self.run()
            best = min(best, time.perf_counter() - t0)
        return best * 1e9


def measure_hw_ns(inputs, reps_hi=9, phases="ABCEF", iters=12):
    """Device time per kernel via repeat-delta: (t[R] - t[1]) / (R - 1)."""
    feats = np.asarray(inputs["feats"])
    H, HID = np.asarray(inputs["attn_l1"]).shape
    cfg = Cfg(N=feats.shape[0], E=np.asarray(inputs["src"]).shape[0],
              F=feats.shape[1], HID=HID, HEADS=H)
    src = np.asarray(inputs["src"], np.int64)
    dst = np.asarray(inputs["dst"], np.int64)
    plan, core_data = make_plan(src, dst, cfg)
    in_maps = make_inputs(inputs, cfg, plan, core_data)
    t = {}
    for reps in (1, reps_hi):
        nc = build_program(cfg, plan, reps=reps, phases=phases,
                           force_shared=True)
        r = Runner(nc, in_maps, cfg.NC)
        t[reps] = r.time_ns(iters=iters)
        del r
    return (t[reps_hi] - t[1]) / (reps_hi - 1)
